# revision 23
# baseline (speedup 1.0000x reference)
"""Trainium2 Bass kernel for nn_MACE (2-layer MACE-style GNN, scalar energy output).

Strategy (8 NeuronCores, SPMD):
  - Edges sharded by destination row range: core c owns nodes [c*6272, (c+1)*6272).
  - Full node-feature table replicated (DRAM, bf16) for the per-edge gather
    (dma_gather, int16 indices; table split at row 25088 into two halves so
    indices fit int16; edges grouped by half into two per-window segments).
  - einsum('eh,ehl->eh') trick: only sum_l rw[:, :, l] is needed, so rw_w3 is
    host-folded to [H, H] — the big radial matmul shrinks 3x.
  - Scatter (segment_sum) via one-hot matmuls: edges row-sorted into 128-node
    windows; per 128-edge tile a selection matrix sel[k, n] = (rel_row[k] == n)
    is built on DVE and PE accumulates agg windows in PSUM.
  - Node-wise linear+LN data-parallel over the core's node slice; updated
    slices are AllGathered (bf16) into the next layer's gather table.
  - Readout reduced per core to one partial scalar; host sums partials.
"""
import math
import sys
from contextlib import ExitStack

import numpy as np
import ml_dtypes

sys.path.insert(0, "/opt/trn_rl_repo")

import concourse.bacc as bacc  # noqa: E402
import concourse.bass as bass  # noqa: E402
import concourse.mybir as mybir  # noqa: E402
import concourse.tile as tile  # noqa: E402
from concourse.bass_utils import run_bass_kernel_spmd  # noqa: E402

AF = mybir.ActivationFunctionType
OP = mybir.AluOpType

N = 50000
E = 800000
H = 128
NB = 8
LMAX = 2
L = 2
CUTOFF = 5.0
NCORES = 8
NPC = 6272                # nodes per core; 8*6272 = 50176 >= N
NPAD = NCORES * NPC
NW = NPC // 128           # 49 windows per core
SPLIT = 25088             # feats table col-split (int16 index limit)
PADV = 1000.0             # rel_row value for padded edge slots (never matches iota)

F32 = mybir.dt.float32
BF16 = mybir.dt.bfloat16
I16 = mybir.dt.int16

MLPC = 384                # radial-MLP chunk: 3 tiles of 128
TPC = MLPC // 128         # tiles per chunk

bf16 = ml_dtypes.bfloat16

SIM_SILU = False   # CoreSim lacks the Silu LUT; emulate via Sigmoid + mult

_CACHE = {}


# ---------------------------------------------------------------- host prep
def _prep(inputs, force_tiles=None):
    row, col = np.asarray(inputs["edge_index"], np.int64)
    pos = np.asarray(inputs["pos"], np.float32)
    an = np.asarray(inputs["atomic_numbers"], np.int64)

    rw_w3 = np.asarray(inputs["rw_w3"], np.float32)     # [L, H, 3H]
    rw_b3 = np.asarray(inputs["rw_b3"], np.float32)     # [L, 3H]
    w3eff = rw_w3.reshape(L, H, H, LMAX + 1).sum(-1)    # [L, H, H]
    b3eff = rw_b3.reshape(L, H, LMAX + 1).sum(-1)       # [L, H]

    feats0 = np.asarray(inputs["node_emb"], np.float32)[an]          # [N, H]
    feats0 = np.concatenate([feats0, np.zeros((NPAD - N, H), np.float32)])
    ae = np.asarray(inputs["ae_emb"], np.float32)[an][:, 0]
    ae = np.concatenate([ae, np.zeros(NPAD - N, np.float32)])

    core_of = row // NPC

    percore = []
    cap_lo = cap_hi = 0
    for c in range(NCORES):
        m = core_of == c
        r = row[m] - c * NPC
        cl = col[m]
        lo = cl < SPLIT
        w = r // 128
        n_lo = np.bincount(w[lo], minlength=NW)
        n_hi = np.bincount(w[~lo], minlength=NW)
        cap_lo = max(cap_lo, int(n_lo.max()))
        cap_hi = max(cap_hi, int(n_hi.max()))
        percore.append((r, cl, lo, w))
    if force_tiles is not None:
        cap_lo = max(cap_lo, force_tiles * 128)
        cap_hi = max(cap_hi, force_tiles * 128)
    # window tile counts, multiples of TPC so MLP chunks align to windows
    tl = -(-cap_lo // 128)
    tl += (-tl) % TPC
    th = -(-cap_hi // 128)
    th += (-th) % TPC
    NSEG_L = NW * tl * 128
    NSEG_H = NW * th * 128
    EPC = NSEG_L + NSEG_H
    T = EPC // 128

    meta = dict(tl=tl, th=th, EPC=EPC, T=T, NSEG_L=NSEG_L, NSEG_H=NSEG_H)

    table0 = feats0.astype(bf16)                     # [NPAD, H]
    roots = (np.arange(1, NB + 1, dtype=np.float32) * math.pi / CUTOFF)

    in_maps = []
    for c in range(NCORES):
        r, cl, lo, w = percore[c]
        ne = len(r)
        # slot assignment: lo-windows first, then hi-windows
        base = np.where(lo, w * (tl * 128), NSEG_L + w * (th * 128))
        order = np.lexsort((np.arange(ne), base))     # stable by window-segment
        rs, cls, bases = r[order], cl[order], base[order]
        # offset within each window-segment
        off = np.arange(ne) - np.searchsorted(bases, bases, side="left")
        slot = bases + off

        filled = np.zeros(EPC, bool)
        filled[slot] = True
        src = np.zeros(EPC, np.int64)
        src[slot] = np.arange(ne)

        posr = pos[np.where(filled, (rs + c * NPC)[src], 0)]
        posc = pos[np.where(filled, np.minimum(cls[src], N - 1), 0)]
        posr[~filled] = 0.0
        posc[~filled] = 0.0
        posc[~filled, 0] = 1.0                        # pad slots get d=1

        relr = np.where(filled, (rs[src] % 128).astype(np.float32), PADV)

        gidx = np.where(filled, cls[src], 0).astype(np.int64)
        idx_lo = np.where(filled[:NSEG_L],
                          np.minimum(gidx[:NSEG_L], SPLIT - 1), 0)
        idx_hi = np.where(filled[NSEG_L:],
                          np.maximum(gidx[NSEG_L:] - SPLIT, 0), 0)

        def wrap16(ix):
            a = ix.astype(np.int16).reshape(-1, 16).T          # [16, n/16]
            return np.ascontiguousarray(np.tile(a, (8, 1)))    # [128, n/16]

        def em(x, dt=np.float32):
            # edge-major channel: slot i -> [i % 128, i // 128, ...]
            x = np.asarray(x, dt)
            tcnt = x.shape[0] // 128
            return np.ascontiguousarray(
                x.reshape(tcnt, 128, *x.shape[1:]).transpose(
                    1, 0, *range(2, x.ndim + 1)))

        nsl = c * NPC
        im = {
            "posr": em(posr), "posc": em(posc), "relr": em(relr, bf16),
            "idx_lo": wrap16(idx_lo), "idx_hi": wrap16(idx_hi),
            "table0": table0,
            "feats_fm0": np.ascontiguousarray(feats0[nsl:nsl + NPC].T),
            "ae_nm": np.ascontiguousarray(
                ae[nsl:nsl + NPC].reshape(NW, 128).T),
            "valid_nm": np.ascontiguousarray(
                (np.arange(nsl, nsl + NPC) < N).astype(np.float32)
                .reshape(NW, 128).T),
            "iden": np.eye(128, dtype=bf16),
            "idenf": np.eye(128, dtype=np.float32),
            "iota": np.tile(np.arange(128, dtype=bf16), (128, 1)).copy(),
            "rootsc": np.tile(roots, (128, 1)).reshape(128, 1, NB).copy(),
            "cbias": np.tile(np.array([0.0, -math.pi, 1e-5], np.float32),
                             (128, 1)),
            "ones_mean": np.full((128, 1), 1.0 / H, np.float32),
            "ones_sum": np.ones((128, 1), np.float32),
            "ones_row": np.ones((1, 128), np.float32),
        }
        for l in range(L):
            im[f"w1_{l}"] = np.asarray(inputs["rw_w1"][l], bf16)
            im[f"b1_{l}"] = np.asarray(
                inputs["rw_b1"][l], np.float32).reshape(128, 1)
            im[f"w2_{l}"] = np.asarray(inputs["rw_w2"][l], bf16)
            im[f"b2_{l}"] = np.asarray(
                inputs["rw_b2"][l], np.float32).reshape(128, 1)
            im[f"w3_{l}"] = np.asarray(w3eff[l], bf16)
            im[f"b3bc_{l}"] = np.tile(
                b3eff[l], (128, 1)).reshape(128, 1, 128).astype(np.float32)
            im[f"linA_{l}"] = np.asarray(inputs["lin_w"][l][:H], np.float32)
            im[f"linB_{l}"] = np.asarray(inputs["lin_w"][l][H:], np.float32)
            im[f"linb_{l}"] = np.asarray(
                inputs["lin_b"][l], np.float32).reshape(128, 1)
            im[f"lng_{l}"] = np.asarray(
                inputs["ln_g"][l], np.float32).reshape(128, 1)
            im[f"lnb_{l}"] = np.asarray(
                inputs["ln_b"][l], np.float32).reshape(128, 1)
        im["row1"] = np.asarray(inputs["ro_w1"], np.float32)
        im["rob1"] = np.asarray(inputs["ro_b1"], np.float32).reshape(128, 1)
        im["row2"] = np.asarray(inputs["ro_w2"], np.float32)
        in_maps.append(im)

    host = dict(
        ro_b2=float(np.asarray(inputs["ro_b2"]).reshape(-1)[0]),
        scale=float(np.asarray(inputs["scale"])),
        shift=float(np.asarray(inputs["shift"])),
    )
    return in_maps, meta, host


# ---------------------------------------------------------------- program
def _build(meta):
    tl, th = meta["tl"], meta["th"]
    EPC, T = meta["EPC"], meta["T"]
    NSEG_L = meta["NSEG_L"]

    nc = bacc.Bacc("TRN2", target_bir_lowering=False, debug=False,
                   num_devices=NCORES)

    def din(name, shape, dt=F32):
        return nc.dram_tensor(name, shape, dt, kind="ExternalInput")

    posr = din("posr", [128, T, 3])
    posc = din("posc", [128, T, 3])
    relr_d = din("relr", [128, T], BF16)
    idx_lo = din("idx_lo", [128, NSEG_L // 16], I16)
    idx_hi = din("idx_hi", [128, (EPC - NSEG_L) // 16], I16)
    table0 = din("table0", [NPAD, H], BF16)
    feats_fm0 = din("feats_fm0", [H, NPC])
    ae_nm = din("ae_nm", [128, NW])
    valid_nm = din("valid_nm", [128, NW])
    iden = din("iden", [128, 128], BF16)
    idenf = din("idenf", [128, 128])
    iota_d = din("iota", [128, 128], BF16)
    rootsc = din("rootsc", [128, 1, NB])
    cbias = din("cbias", [128, 3])
    ones_mean = din("ones_mean", [128, 1])
    ones_sum = din("ones_sum", [128, 1])
    ones_row = din("ones_row", [1, 128])

    wts = {}
    for l in range(L):
        wts[f"w1_{l}"] = din(f"w1_{l}", [NB, 128], BF16)
        wts[f"b1_{l}"] = din(f"b1_{l}", [128, 1])
        wts[f"w2_{l}"] = din(f"w2_{l}", [128, 128], BF16)
        wts[f"b2_{l}"] = din(f"b2_{l}", [128, 1])
        wts[f"w3_{l}"] = din(f"w3_{l}", [128, 128], BF16)
        wts[f"b3bc_{l}"] = din(f"b3bc_{l}", [128, 1, 128])
        wts[f"linA_{l}"] = din(f"linA_{l}", [128, 128])
        wts[f"linB_{l}"] = din(f"linB_{l}", [128, 128])
        wts[f"linb_{l}"] = din(f"linb_{l}", [128, 1])
        wts[f"lng_{l}"] = din(f"lng_{l}", [128, 1])
        wts[f"lnb_{l}"] = din(f"lnb_{l}", [128, 1])
    row1 = din("row1", [128, 128])
    rob1 = din("rob1", [128, 1])
    row2 = din("row2", [128, 1])

    out = nc.dram_tensor("out", [1, 1], F32, kind="ExternalOutput")

    with tile.TileContext(nc) as tc, ExitStack() as ctx:
        dram = ctx.enter_context(tc.tile_pool(name="dram", bufs=1, space="DRAM"))
        rbf_dram = dram.tile([NB, EPC], BF16)
        ag_in = dram.tile([NPC, H], BF16)
        table1 = nc.dram_tensor("table1", [NPAD, H], BF16, addr_space="Shared")

        cpool = ctx.enter_context(tc.tile_pool(name="consts", bufs=1))

        def cload(ap):
            t = cpool.tile(list(ap.shape), ap.dtype, tag=f"c_{ap.tensor.name}")
            nc.sync.dma_start(t[:], ap)
            return t

        relr_s = cload(relr_d.ap())
        idxl_s = cload(idx_lo.ap())
        idxh_s = cload(idx_hi.ap())
        iden_s = cload(iden.ap())
        idenf_s = cload(idenf.ap())
        iota_s = cload(iota_d.ap())
        rootsc_s = cload(rootsc.ap())
        cb_s = cload(cbias.ap())
        for ci_, cv_ in enumerate([0.0, -math.pi, 1e-5]):
            nc.const_aps.aps[(F32, cv_)] = cb_s[:, ci_:ci_ + 1]
        onesm_s = cload(ones_mean.ap())
        oness_s = cload(ones_sum.ap())
        onesr_s = cload(ones_row.ap())
        ae_s = cload(ae_nm.ap())
        valid_s = cload(valid_nm.ap())
        wt_s = {k: cload(v.ap()) for k, v in wts.items()}
        row1_s = cload(row1.ap())
        rob1_s = cload(rob1.ap())
        row2_s = cload(row2.ap())

        feats_fm = cpool.tile([H, NPC], F32, tag="feats_fm")
        nc.sync.dma_start(feats_fm[:], feats_fm0.ap())
        agg = cpool.tile([H, NPC], F32, tag="agg")

        def act_silu(pool, out_t, in_ap, bias_ap, tag):
            if not SIM_SILU:
                nc.scalar.activation(out_t[:], in_ap, AF.Silu, bias=bias_ap)
            else:
                shp = list(in_ap.shape)
                xt = pool.tile(shp, F32, tag=f"{tag}_x")
                nc.scalar.activation(xt[:], in_ap, AF.Identity, bias=bias_ap)
                sg = pool.tile(shp, F32, tag=f"{tag}_s")
                nc.scalar.activation(sg[:], xt[:], AF.Sigmoid)
                nc.vector.tensor_tensor(out=out_t[:], in0=xt[:], in1=sg[:],
                                        op=OP.mult)

        # ---------------- phase R: radial basis, once ----------------
        with tc.tile_pool(name="rbfp", bufs=1) as rp, \
             tc.tile_pool(name="rbfw", bufs=2) as rw_, \
             tc.tile_pool(name="rbfps", bufs=2, space="PSUM") as rpp:
            pr = rp.tile([128, T, 3], F32, tag="pr")
            pc = rp.tile([128, T, 3], F32, tag="pc")
            nc.sync.dma_start(pr[:], posr.ap())
            nc.sync.dma_start(pc[:], posc.ap())
            dx = rp.tile([128, T, 3], F32, tag="dx")
            nc.vector.tensor_tensor(out=dx[:], in0=pc[:], in1=pr[:],
                                    op=OP.subtract)
            nc.vector.tensor_tensor(out=dx[:], in0=dx[:], in1=dx[:], op=OP.mult)
            d2 = rp.tile([128, T], F32, tag="d2")
            nc.vector.tensor_reduce(out=d2[:], in_=dx[:],
                                    axis=mybir.AxisListType.X, op=OP.add)
            dd = rp.tile([128, T], F32, tag="dd")
            nc.scalar.activation(dd[:], d2[:], AF.Sqrt)
            # negated envelope: -0.5*(cos(d*pi/C)+1) = sin(d*pi/(2C))^2 - 1
            co = rp.tile([128, T], F32, tag="co")
            nc.scalar.activation(co[:], dd[:], AF.Sin,
                                 scale=math.pi / (2 * CUTOFF))
            nc.scalar.activation(co[:], co[:], AF.Square)
            nc.vector.tensor_scalar(out=co[:], in0=co[:], scalar1=1.0,
                                    scalar2=None, op0=OP.subtract)
            msk = rp.tile([128, T], F32, tag="msk")
            nc.vector.tensor_scalar(out=msk[:], in0=dd[:],
                                    scalar1=float(CUTOFF), scalar2=None,
                                    op0=OP.is_lt)
            nc.vector.tensor_tensor(out=co[:], in0=co[:], in1=msk[:],
                                    op=OP.mult)
            dcl = rp.tile([128, T], F32, tag="dcl")
            nc.vector.tensor_scalar(out=dcl[:], in0=dd[:], scalar1=1e-3,
                                    scalar2=None, op0=OP.max)
            rec = rp.tile([128, T], F32, tag="rec")
            nc.vector.reciprocal(rec[:], dcl[:])
            nc.vector.tensor_tensor(out=co[:], in0=co[:], in1=rec[:],
                                    op=OP.mult)
            # x = d*root_b >= 0; reduce to y in [0, 2pi) by conditional
            # subtraction, then sin(y - pi) = -sin(x); the negated envelope
            # cancels the sign.
            rb = rp.tile([128, T, NB], F32, tag="rb")
            nc.vector.tensor_tensor(
                out=rb[:], in0=dd[:, :, None].to_broadcast([128, T, NB]),
                in1=rootsc_s[:].to_broadcast([128, T, NB]), op=OP.mult)
            sub = rp.tile([128, T, NB], F32, tag="sub")
            for c in (8 * math.pi, 4 * math.pi, 2 * math.pi):
                nc.vector.tensor_scalar(out=sub[:], in0=rb[:],
                                        scalar1=float(c), scalar2=float(c),
                                        op0=OP.is_ge, op1=OP.mult)
                nc.vector.tensor_tensor(out=rb[:], in0=rb[:], in1=sub[:],
                                        op=OP.subtract)
            tau_lo = float(np.nextafter(np.float32(2 * math.pi), np.float32(0)))
            nc.vector.tensor_scalar(out=rb[:], in0=rb[:], scalar1=tau_lo,
                                    scalar2=None, op0=OP.min)
            nc.scalar.activation(rb[:], rb[:], AF.Sin, bias=-math.pi)
            rbb = rp.tile([128, T, NB], BF16, tag="rbb")
            nc.vector.tensor_tensor(
                out=rbb[:], in0=rb[:],
                in1=co[:, :, None].to_broadcast([128, T, NB]), op=OP.mult)
            ngrp = -(-T // 16)
            for g in range(ngrp):
                t0 = 16 * g
                tn = min(16, T - t0)
                tp = rpp.tile([128, 128], BF16, tag="tp")
                nc.tensor.transpose(tp[:tn * NB, :],
                                    rbb[:, t0:t0 + tn, :], iden_s[:])
                tsb = rw_.tile([128, 128], BF16, tag="tsb")
                nc.vector.tensor_copy(out=tsb[:tn * NB, :], in_=tp[:tn * NB, :])
                # dram[b, 128*(t0+t'') + e] <- tsb[NB*t'' + b, e]
                base = rbf_dram[:]
                dram_ap = bass.AP(base.tensor, base.offset + 128 * t0,
                                  [[128, tn], [EPC, NB], [1, 128]])
                nc.sync.dma_start(dram_ap, tsb[:tn * NB, :])

        # ---------------- main pools ----------------
        gp = ctx.enter_context(tc.tile_pool(name="gath", bufs=3))
        mp = ctx.enter_context(tc.tile_pool(name="mlp", bufs=3))
        pp = ctx.enter_context(tc.tile_pool(name="mlpp", bufs=3, space="PSUM"))
        wpp = ctx.enter_context(tc.tile_pool(name="winp", bufs=2, space="PSUM"))
        npool = ctx.enter_context(tc.tile_pool(name="node", bufs=2))
        npp = ctx.enter_context(tc.tile_pool(name="nodep", bufs=2, space="PSUM"))
        spp = ctx.enter_context(tc.tile_pool(name="statp", bufs=1, space="PSUM"))

        def layer(l, tab_lo, tab_hi):
            def seg(t_off, wtiles, idxs, tab, first):
                cpw = wtiles // TPC
                for w in range(NW):
                    npos = wtiles * 128
                    pos0 = t_off * 128 + w * npos   # global edge position
                    spos = w * npos                 # segment-local position
                    nj = gp.tile([128, wtiles, H], BF16, tag="gat")
                    nc.gpsimd.dma_gather(
                        nj[:], tab, idxs[:, spos // 16:(spos + npos) // 16],
                        npos, npos, H, single_packet=(npos <= 1024))
                    wps = wpp.tile([128, 128], F32, tag="wps")
                    for cc in range(cpw):
                        e0 = pos0 + cc * MLPC
                        rbfc = mp.tile([NB, MLPC], BF16, tag="rbfc")
                        nc.sync.dma_start(rbfc[:], rbf_dram[:, e0:e0 + MLPC])
                        h1p = pp.tile([128, MLPC], F32, tag="mps")
                        nc.tensor.matmul(h1p[:], lhsT=wt_s[f"w1_{l}"][:],
                                         rhs=rbfc[:], start=True, stop=True)
                        h1 = mp.tile([128, MLPC], BF16, tag="h1")
                        act_silu(mp, h1, h1p[:], wt_s[f"b1_{l}"][:, 0:1], "h1")
                        h2p = pp.tile([128, MLPC], F32, tag="mps")
                        nc.tensor.matmul(h2p[:], lhsT=wt_s[f"w2_{l}"][:],
                                         rhs=h1[:], start=True, stop=True)
                        h2 = mp.tile([128, MLPC], BF16, tag="h2")
                        act_silu(mp, h2, h2p[:], wt_s[f"b2_{l}"][:, 0:1], "h2")
                        rwp = pp.tile([128, TPC, 128], F32, tag="mps")
                        for k in range(TPC):
                            nc.tensor.matmul(rwp[:, k, :],
                                             lhsT=h2[:, 128 * k:128 * (k + 1)],
                                             rhs=wt_s[f"w3_{l}"][:],
                                             start=True, stop=True)
                        rwb = mp.tile([128, TPC, 128], BF16, tag="rwb")
                        nc.vector.tensor_tensor(
                            out=rwb[:], in0=rwp[:],
                            in1=wt_s[f"b3bc_{l}"][:].to_broadcast(
                                [128, TPC, 128]), op=OP.add)
                        for k in range(TPC):
                            j = cc * TPC + k
                            tglob = t_off + w * wtiles + j
                            msgs = mp.tile([128, 128], BF16, tag="msgs")
                            nc.vector.tensor_tensor(
                                out=msgs[:], in0=nj[:, j, :], in1=rwb[:, k, :],
                                op=OP.mult)
                            sel = mp.tile([128, 128], BF16, tag="sel")
                            nc.vector.tensor_tensor(
                                out=sel[:],
                                in0=relr_s[:, tglob:tglob + 1]
                                .to_broadcast([128, 128]),
                                in1=iota_s[:], op=OP.is_equal)
                            nc.tensor.matmul(wps[:], lhsT=msgs[:], rhs=sel[:],
                                             start=(j == 0),
                                             stop=(j == wtiles - 1))
                    if first:
                        nc.vector.tensor_copy(
                            out=agg[:, 128 * w:128 * (w + 1)], in_=wps[:])
                    else:
                        nc.vector.tensor_tensor(
                            out=agg[:, 128 * w:128 * (w + 1)],
                            in0=agg[:, 128 * w:128 * (w + 1)], in1=wps[:],
                            op=OP.add)

            seg(0, tl, idxl_s[:], tab_lo, True)
            seg(NSEG_L // 128, th, idxh_s[:], tab_hi, False)

            # node update + LN per window (feature-major)
            for w in range(NW):
                sl = slice(128 * w, 128 * (w + 1))
                up = npp.tile([128, 128], F32, tag="np1")
                nc.tensor.matmul(up[:], lhsT=wt_s[f"linA_{l}"][:],
                                 rhs=feats_fm[:, sl], start=True, stop=False)
                nc.tensor.matmul(up[:], lhsT=wt_s[f"linB_{l}"][:],
                                 rhs=agg[:, sl], start=False, stop=True)
                x = npool.tile([128, 128], F32, tag="x")
                nc.vector.tensor_tensor(out=x[:], in0=up[:],
                                        in1=feats_fm[:, sl], op=OP.add)
                nc.vector.tensor_scalar(out=x[:], in0=x[:],
                                        scalar1=wt_s[f"linb_{l}"][:, 0:1],
                                        scalar2=None, op0=OP.add)
                x2 = npool.tile([128, 128], F32, tag="x2")
                nc.vector.tensor_tensor(out=x2[:], in0=x[:], in1=x[:],
                                        op=OP.mult)
                st = spp.tile([1, 256], F32, tag="st")
                nc.tensor.matmul(st[:, 0:128], lhsT=onesm_s[:], rhs=x[:],
                                 start=True, stop=True)
                nc.tensor.matmul(st[:, 128:256], lhsT=onesm_s[:], rhs=x2[:],
                                 start=True, stop=True)
                stv = npool.tile([1, 256], F32, tag="stv")
                nc.vector.tensor_copy(out=stv[:], in_=st[:])
                var = npool.tile([1, 128], F32, tag="var")
                nc.vector.tensor_tensor(out=var[:], in0=stv[:, 0:128],
                                        in1=stv[:, 0:128], op=OP.mult)
                nc.vector.tensor_tensor(out=var[:], in0=stv[:, 128:256],
                                        in1=var[:], op=OP.subtract)
                sd = npool.tile([1, 128], F32, tag="sd")
                nc.scalar.activation(sd[:], var[:], AF.Sqrt, bias=1e-5)
                rs = npool.tile([1, 128], F32, tag="rs")
                nc.vector.reciprocal(rs[:], sd[:])
                mr = npool.tile([1, 128], F32, tag="mr")
                nc.vector.tensor_tensor(out=mr[:], in0=stv[:, 0:128],
                                        in1=rs[:], op=OP.mult)
                bc = npp.tile([128, 256], F32, tag="np1")
                nc.tensor.matmul(bc[:, 0:128], lhsT=onesr_s[:], rhs=rs[:],
                                 start=True, stop=True)
                nc.tensor.matmul(bc[:, 128:256], lhsT=onesr_s[:], rhs=mr[:],
                                 start=True, stop=True)
                xn = npool.tile([128, 128], F32, tag="xn")
                nc.vector.tensor_tensor(out=xn[:], in0=x[:], in1=bc[:, 0:128],
                                        op=OP.mult)
                nc.vector.tensor_tensor(out=xn[:], in0=xn[:],
                                        in1=bc[:, 128:256], op=OP.subtract)
                nc.vector.tensor_scalar(out=feats_fm[:, sl], in0=xn[:],
                                        scalar1=wt_s[f"lng_{l}"][:, 0:1],
                                        scalar2=wt_s[f"lnb_{l}"][:, 0:1],
                                        op0=OP.mult, op1=OP.add)
                if l == 0:
                    tpn = npp.tile([128, 128], F32, tag="np1")
                    nc.tensor.transpose(tpn[:], feats_fm[:, sl], idenf_s[:])
                    nm = npool.tile([128, 128], BF16, tag="nm")
                    nc.vector.tensor_copy(out=nm[:], in_=tpn[:])
                    nc.sync.dma_start(ag_in[sl, :], nm[:])

        layer(0, table0.ap()[0:SPLIT, :], table0.ap()[SPLIT:NPAD, :])
        nc.gpsimd.collective_compute(
            "AllGather", OP.bypass,
            replica_groups=[list(range(NCORES))],
            ins=[ag_in.opt()], outs=[table1.ap().opt()])
        layer(1, table1.ap()[0:SPLIT, :], table1.ap()[SPLIT:NPAD, :])

        # ---------------- readout ----------------
        er = cpool.tile([128, NW], F32, tag="er")
        for w in range(NW):
            sl = slice(128 * w, 128 * (w + 1))
            ap_ = npp.tile([128, 128], F32, tag="np1")
            nc.tensor.matmul(ap_[:], lhsT=row1_s[:], rhs=feats_fm[:, sl],
                             start=True, stop=True)
            a = npool.tile([128, 128], F32, tag="a")
            act_silu(npool, a, ap_[:], rob1_s[:, 0:1], "a")
            ep = npp.tile([128, 1], F32, tag="np1")
            nc.tensor.matmul(ep[:], lhsT=a[:], rhs=row2_s[:, 0:1],
                             start=True, stop=True)
            nc.vector.tensor_copy(out=er[:, w:w + 1], in_=ep[:])
        nc.vector.tensor_tensor(out=er[:], in0=er[:], in1=ae_s[:], op=OP.add)
        nc.vector.tensor_tensor(out=er[:], in0=er[:], in1=valid_s[:],
                                op=OP.mult)
        erd = cpool.tile([128, 1], F32, tag="erd")
        nc.vector.tensor_reduce(out=erd[:], in_=er[:],
                                axis=mybir.AxisListType.X, op=OP.add)
        tot = spp.tile([1, 1], F32, tag="st")
        nc.tensor.matmul(tot[:], lhsT=oness_s[:], rhs=erd[:],
                         start=True, stop=True)
        tsb1 = cpool.tile([1, 1], F32, tag="tsb1")
        nc.vector.tensor_copy(out=tsb1[:], in_=tot[:])
        nc.sync.dma_start(out.ap(), tsb1[:])

    nc.compile()
    return nc


# ---------------------------------------------------------------- entry
def kernel(**inputs):
    in_maps, meta, host = _prep(inputs)
    key = tuple(sorted(meta.items()))
    if key not in _CACHE:
        _CACHE[key] = _build(meta)
    nc = _CACHE[key]
    res = run_bass_kernel_spmd(nc, in_maps, core_ids=list(range(NCORES)))
    partials = [float(r["out"][0, 0]) for r in res.results]
    # device readout omits the per-node ro_b2 constant; add it for valid nodes
    total = sum(partials) + host["ro_b2"] * N
    return np.float32(total * host["scale"] + host["shift"])


# revision 38
# speedup vs baseline: 1.1429x; 1.1429x over previous
"""Trainium2 Bass kernel for nn_MACE (2-layer MACE-style GNN, scalar energy output).

Strategy (8 NeuronCores, SPMD):
  - Edges sharded by destination row range: core c owns nodes [c*6272, (c+1)*6272).
  - Full node-feature table replicated (DRAM, bf16) for the per-edge gather
    (dma_gather, int16 indices; table split at row 25088 into two halves so
    indices fit int16; edges grouped by half into two per-window segments).
  - einsum('eh,ehl->eh') trick: only sum_l rw[:, :, l] is needed, so rw_w3 is
    host-folded to [H, H] — the big radial matmul shrinks 3x.
  - Scatter (segment_sum) via one-hot matmuls: edges row-sorted into 128-node
    windows; per 128-edge tile a selection matrix sel[k, n] = (rel_row[k] == n)
    is built on DVE and PE accumulates agg windows in PSUM.
  - Node-wise linear+LN data-parallel over the core's node slice; updated
    slices are AllGathered (bf16) into the next layer's gather table.
  - Readout reduced per core to one partial scalar; host sums partials.
"""
import math
import sys
from contextlib import ExitStack

import numpy as np
import ml_dtypes

sys.path.insert(0, "/opt/trn_rl_repo")

import concourse.bacc as bacc  # noqa: E402
import concourse.bass as bass  # noqa: E402
import concourse.mybir as mybir  # noqa: E402
import concourse.tile as tile  # noqa: E402
from concourse.bass_utils import run_bass_kernel_spmd  # noqa: E402

AF = mybir.ActivationFunctionType
OP = mybir.AluOpType

N = 50000
E = 800000
H = 128
NB = 8
LMAX = 2
L = 2
CUTOFF = 5.0
NCORES = 8
NPC = 6272                # nodes per core; 8*6272 = 50176 >= N
NPAD = NCORES * NPC
NW = NPC // 128           # 49 windows per core
SPLIT = 25088             # feats table col-split (int16 index limit)
PADV = 1000.0             # rel_row value for padded edge slots (never matches iota)

F32 = mybir.dt.float32
BF16 = mybir.dt.bfloat16
I16 = mybir.dt.int16

MLPC = 384                # radial-MLP chunk: 3 tiles of 128
TPC = MLPC // 128         # tiles per chunk

bf16 = ml_dtypes.bfloat16

SIM_SILU = False   # CoreSim lacks the Silu LUT; emulate via Sigmoid + mult

_CACHE = {}


# ---------------------------------------------------------------- host prep
def _prep(inputs, force_tiles=None):
    row, col = np.asarray(inputs["edge_index"], np.int64)
    pos = np.asarray(inputs["pos"], np.float32)
    an = np.asarray(inputs["atomic_numbers"], np.int64)

    rw_w3 = np.asarray(inputs["rw_w3"], np.float32)     # [L, H, 3H]
    rw_b3 = np.asarray(inputs["rw_b3"], np.float32)     # [L, 3H]
    w3eff = rw_w3.reshape(L, H, H, LMAX + 1).sum(-1)    # [L, H, H]
    b3eff = rw_b3.reshape(L, H, LMAX + 1).sum(-1)       # [L, H]

    feats0 = np.asarray(inputs["node_emb"], np.float32)[an]          # [N, H]
    feats0 = np.concatenate([feats0, np.zeros((NPAD - N, H), np.float32)])
    ae = np.asarray(inputs["ae_emb"], np.float32)[an][:, 0]
    ae = np.concatenate([ae, np.zeros(NPAD - N, np.float32)])

    core_of = row // NPC

    percore = []
    cap_lo = cap_hi = 0
    for c in range(NCORES):
        m = core_of == c
        r = row[m] - c * NPC
        cl = col[m]
        lo = cl < SPLIT
        w = r // 128
        n_lo = np.bincount(w[lo], minlength=NW)
        n_hi = np.bincount(w[~lo], minlength=NW)
        cap_lo = max(cap_lo, int(n_lo.max()))
        cap_hi = max(cap_hi, int(n_hi.max()))
        percore.append((r, cl, lo, w))
    if force_tiles is not None:
        cap_lo = max(cap_lo, force_tiles * 128)
        cap_hi = max(cap_hi, force_tiles * 128)
    # window tile counts, multiples of TPC so MLP chunks align to windows
    tl = -(-cap_lo // 128)
    tl += (-tl) % TPC
    th = -(-cap_hi // 128)
    th += (-th) % TPC
    NSEG_L = NW * tl * 128
    NSEG_H = NW * th * 128
    EPC = NSEG_L + NSEG_H
    T = EPC // 128

    meta = dict(tl=tl, th=th, EPC=EPC, T=T, NSEG_L=NSEG_L, NSEG_H=NSEG_H)

    table0 = feats0.astype(bf16)                     # [NPAD, H]
    roots = (np.arange(1, NB + 1, dtype=np.float32) * math.pi / CUTOFF)

    in_maps = []
    for c in range(NCORES):
        r, cl, lo, w = percore[c]
        ne = len(r)
        # slot assignment: lo-windows first, then hi-windows
        base = np.where(lo, w * (tl * 128), NSEG_L + w * (th * 128))
        order = np.lexsort((np.arange(ne), base))     # stable by window-segment
        rs, cls, bases = r[order], cl[order], base[order]
        # offset within each window-segment
        off = np.arange(ne) - np.searchsorted(bases, bases, side="left")
        slot = bases + off

        filled = np.zeros(EPC, bool)
        filled[slot] = True
        src = np.zeros(EPC, np.int64)
        src[slot] = np.arange(ne)

        posr = pos[np.where(filled, (rs + c * NPC)[src], 0)]
        posc = pos[np.where(filled, np.minimum(cls[src], N - 1), 0)]
        posr[~filled] = 0.0
        posc[~filled] = 0.0
        posc[~filled, 0] = 1.0                        # pad slots get d=1

        relr = np.where(filled, (rs[src] % 128).astype(np.float32), PADV)

        gidx = np.where(filled, cls[src], 0).astype(np.int64)
        idx_lo = np.where(filled[:NSEG_L],
                          np.minimum(gidx[:NSEG_L], SPLIT - 1), 0)
        idx_hi = np.where(filled[NSEG_L:],
                          np.maximum(gidx[NSEG_L:] - SPLIT, 0), 0)

        def wrap16(ix):
            a = ix.astype(np.int16).reshape(-1, 16).T          # [16, n/16]
            return np.ascontiguousarray(np.tile(a, (8, 1)))    # [128, n/16]

        def em(x, dt=np.float32):
            # edge-major channel: slot i -> [i % 128, i // 128, ...]
            x = np.asarray(x, dt)
            tcnt = x.shape[0] // 128
            return np.ascontiguousarray(
                x.reshape(tcnt, 128, *x.shape[1:]).transpose(
                    1, 0, *range(2, x.ndim + 1)))

        nsl = c * NPC
        im = {
            "posr": em(posr), "posc": em(posc),
            "relr": em(relr, bf16).reshape(128, -1, 1).copy(),
            "idx_lo": wrap16(idx_lo), "idx_hi": wrap16(idx_hi),
            "table0": table0,
            "feats_fm0": np.ascontiguousarray(feats0[nsl:nsl + NPC].T),
            "ae_nm": np.ascontiguousarray(
                ae[nsl:nsl + NPC].reshape(NW, 128).T),
            "valid_nm": np.ascontiguousarray(
                (np.arange(nsl, nsl + NPC) < N).astype(np.float32)
                .reshape(NW, 128).T),
            "iden": np.eye(128, dtype=bf16),
            "idenf": np.eye(128, dtype=np.float32),
            "iota": np.tile(np.arange(128, dtype=bf16),
                            (128, 1)).reshape(128, 1, 128).copy(),
            "rootsc": np.tile(roots, (128, 1)).reshape(128, 1, NB).copy(),
            "cbias": np.tile(np.array([0.0, -math.pi, 1e-5], np.float32),
                             (128, 1)),
            "ones_mean": np.full((128, 1), 1.0 / H, np.float32),
            "ones_sum": np.ones((128, 1), np.float32),
            "ones_row": np.ones((1, 128), np.float32),
        }
        for l in range(L):
            im[f"w1_{l}"] = np.asarray(inputs["rw_w1"][l], bf16)
            im[f"b1_{l}"] = np.asarray(
                inputs["rw_b1"][l], np.float32).reshape(128, 1)
            im[f"w2_{l}"] = np.asarray(inputs["rw_w2"][l], bf16)
            im[f"b2_{l}"] = np.asarray(
                inputs["rw_b2"][l], np.float32).reshape(128, 1)
            im[f"w3_{l}"] = np.asarray(w3eff[l], bf16)
            im[f"b3bc_{l}"] = np.tile(
                b3eff[l], (128, 1)).reshape(128, 1, 128).astype(np.float32)
            im[f"linA_{l}"] = np.asarray(inputs["lin_w"][l][:H], np.float32)
            im[f"linB_{l}"] = np.asarray(inputs["lin_w"][l][H:], np.float32)
            im[f"linb_{l}"] = np.asarray(
                inputs["lin_b"][l], np.float32).reshape(1, 128)
            im[f"lng_{l}"] = np.asarray(
                inputs["ln_g"][l], np.float32).reshape(128, 1)
            im[f"lnb_{l}"] = np.asarray(
                inputs["ln_b"][l], np.float32).reshape(128, 1)
        im["row1"] = np.asarray(inputs["ro_w1"], np.float32)
        im["rob1"] = np.asarray(inputs["ro_b1"], np.float32).reshape(128, 1)
        im["row2"] = np.asarray(inputs["ro_w2"], np.float32)
        in_maps.append(im)

    host = dict(
        ro_b2=float(np.asarray(inputs["ro_b2"]).reshape(-1)[0]),
        scale=float(np.asarray(inputs["scale"])),
        shift=float(np.asarray(inputs["shift"])),
    )
    return in_maps, meta, host


# ---------------------------------------------------------------- program
def _build(meta):
    tl, th = meta["tl"], meta["th"]
    EPC, T = meta["EPC"], meta["T"]
    NSEG_L = meta["NSEG_L"]

    nc = bacc.Bacc("TRN2", target_bir_lowering=False, debug=False,
                   num_devices=NCORES, num_swdge_queues=4)

    def din(name, shape, dt=F32):
        return nc.dram_tensor(name, shape, dt, kind="ExternalInput")

    posr = din("posr", [128, T, 3])
    posc = din("posc", [128, T, 3])
    relr_d = din("relr", [128, T, 1], BF16)
    idx_lo = din("idx_lo", [128, NSEG_L // 16], I16)
    idx_hi = din("idx_hi", [128, (EPC - NSEG_L) // 16], I16)
    table0 = din("table0", [NPAD, H], BF16)
    feats_fm0 = din("feats_fm0", [H, NPC])
    ae_nm = din("ae_nm", [128, NW])
    valid_nm = din("valid_nm", [128, NW])
    iden = din("iden", [128, 128], BF16)
    idenf = din("idenf", [128, 128])
    iota_d = din("iota", [128, 1, 128], BF16)
    rootsc = din("rootsc", [128, 1, NB])
    cbias = din("cbias", [128, 3])
    ones_mean = din("ones_mean", [128, 1])
    ones_sum = din("ones_sum", [128, 1])
    ones_row = din("ones_row", [1, 128])

    wts = {}
    for l in range(L):
        wts[f"w1_{l}"] = din(f"w1_{l}", [NB, 128], BF16)
        wts[f"b1_{l}"] = din(f"b1_{l}", [128, 1])
        wts[f"w2_{l}"] = din(f"w2_{l}", [128, 128], BF16)
        wts[f"b2_{l}"] = din(f"b2_{l}", [128, 1])
        wts[f"w3_{l}"] = din(f"w3_{l}", [128, 128], BF16)
        wts[f"b3bc_{l}"] = din(f"b3bc_{l}", [128, 1, 128])
        wts[f"linA_{l}"] = din(f"linA_{l}", [128, 128])
        wts[f"linB_{l}"] = din(f"linB_{l}", [128, 128])
        wts[f"linb_{l}"] = din(f"linb_{l}", [1, 128])
        wts[f"lng_{l}"] = din(f"lng_{l}", [128, 1])
        wts[f"lnb_{l}"] = din(f"lnb_{l}", [128, 1])
    row1 = din("row1", [128, 128])
    rob1 = din("rob1", [128, 1])
    row2 = din("row2", [128, 1])

    out = nc.dram_tensor("out", [1, 1], F32, kind="ExternalOutput")

    with tile.TileContext(nc) as tc, ExitStack() as ctx:
        dram = ctx.enter_context(tc.tile_pool(name="dram", bufs=1, space="DRAM"))
        rbf_dram = dram.tile([NB, EPC], BF16)
        ag_in = dram.tile([NPC, H], BF16)
        table1 = nc.dram_tensor("table1", [NPAD, H], BF16, addr_space="Shared")

        cpool = ctx.enter_context(tc.tile_pool(name="consts", bufs=1))

        def cload(ap):
            t = cpool.tile(list(ap.shape), ap.dtype, tag=f"c_{ap.tensor.name}")
            nc.sync.dma_start(t[:], ap)
            return t

        relr_s = cload(relr_d.ap())
        idxl_s = cload(idx_lo.ap())
        idxh_s = cload(idx_hi.ap())
        iden_s = cload(iden.ap())
        idenf_s = cload(idenf.ap())
        iota_s = cload(iota_d.ap())
        rootsc_s = cload(rootsc.ap())
        cb_s = cload(cbias.ap())
        for ci_, cv_ in enumerate([0.0, -math.pi, 1e-5]):
            nc.const_aps.aps[(F32, cv_)] = cb_s[:, ci_:ci_ + 1]
        onesm_s = cload(ones_mean.ap())
        oness_s = cload(ones_sum.ap())
        onesr_s = cload(ones_row.ap())
        ae_s = cload(ae_nm.ap())
        valid_s = cload(valid_nm.ap())
        wt_s = {k: cload(v.ap()) for k, v in wts.items()}
        row1_s = cload(row1.ap())
        rob1_s = cload(rob1.ap())
        row2_s = cload(row2.ap())

        feats_fm = cpool.tile([H, NPC], F32, tag="feats_fm")
        nc.sync.dma_start(feats_fm[:], feats_fm0.ap())
        agg = cpool.tile([H, NPC], F32, tag="agg")

        def act_silu(pool, out_t, in_ap, bias_ap, tag):
            if not SIM_SILU:
                nc.scalar.activation(out_t[:], in_ap, AF.Silu, bias=bias_ap)
            else:
                shp = list(in_ap.shape)
                xt = pool.tile(shp, F32, tag=f"{tag}_x")
                nc.scalar.activation(xt[:], in_ap, AF.Identity, bias=bias_ap)
                sg = pool.tile(shp, F32, tag=f"{tag}_s")
                nc.scalar.activation(sg[:], xt[:], AF.Sigmoid)
                nc.vector.tensor_tensor(out=out_t[:], in0=xt[:], in1=sg[:],
                                        op=OP.mult)

        # ---------------- phase R: radial basis, once (chunked) ----------
        with tc.tile_pool(name="rbfp", bufs=1) as rp, \
             tc.tile_pool(name="rbfw", bufs=2) as rw_, \
             tc.tile_pool(name="rbfps", bufs=2, space="PSUM") as rpp:
            TC = -(-T // 2)
            for tc0 in range(0, T, TC):
                tcn = min(TC, T - tc0)
                pr = rp.tile([128, TC, 3], F32, tag="pr")
                pc = rp.tile([128, TC, 3], F32, tag="pc")
                nc.sync.dma_start(pr[:, :tcn, :], posr.ap()[:, tc0:tc0 + tcn, :])
                nc.sync.dma_start(pc[:, :tcn, :], posc.ap()[:, tc0:tc0 + tcn, :])
                dx = rp.tile([128, TC, 3], F32, tag="dx")
                nc.vector.tensor_tensor(out=dx[:, :tcn, :], in0=pc[:, :tcn, :],
                                        in1=pr[:, :tcn, :], op=OP.subtract)
                nc.vector.tensor_tensor(out=dx[:, :tcn, :], in0=dx[:, :tcn, :],
                                        in1=dx[:, :tcn, :], op=OP.mult)
                d2 = rp.tile([128, TC], F32, tag="d2")
                nc.vector.tensor_reduce(out=d2[:, :tcn], in_=dx[:, :tcn, :],
                                        axis=mybir.AxisListType.X, op=OP.add)
                dd = rp.tile([128, TC], F32, tag="dd")
                nc.scalar.activation(dd[:, :tcn], d2[:, :tcn], AF.Sqrt)
                # negated envelope: -0.5*(cos(d*pi/C)+1) = sin(d*pi/(2C))^2 - 1
                co = rp.tile([128, TC], F32, tag="co")
                nc.scalar.activation(co[:, :tcn], dd[:, :tcn], AF.Sin,
                                     scale=math.pi / (2 * CUTOFF))
                nc.scalar.activation(co[:, :tcn], co[:, :tcn], AF.Square)
                nc.vector.tensor_scalar(out=co[:, :tcn], in0=co[:, :tcn],
                                        scalar1=1.0, scalar2=None,
                                        op0=OP.subtract)
                msk = rp.tile([128, TC], F32, tag="msk")
                nc.vector.tensor_scalar(out=msk[:, :tcn], in0=dd[:, :tcn],
                                        scalar1=float(CUTOFF), scalar2=None,
                                        op0=OP.is_lt)
                nc.vector.tensor_tensor(out=co[:, :tcn], in0=co[:, :tcn],
                                        in1=msk[:, :tcn], op=OP.mult)
                dcl = rp.tile([128, TC], F32, tag="dcl")
                nc.vector.tensor_scalar(out=dcl[:, :tcn], in0=dd[:, :tcn],
                                        scalar1=1e-3, scalar2=None, op0=OP.max)
                rec = rp.tile([128, TC], F32, tag="rec")
                nc.vector.reciprocal(rec[:, :tcn], dcl[:, :tcn])
                nc.vector.tensor_tensor(out=co[:, :tcn], in0=co[:, :tcn],
                                        in1=rec[:, :tcn], op=OP.mult)
                rb = rp.tile([128, TC, NB], F32, tag="rb")
                nc.vector.tensor_tensor(
                    out=rb[:, :tcn, :],
                    in0=dd[:, :tcn, None].to_broadcast([128, tcn, NB]),
                    in1=rootsc_s[:].to_broadcast([128, tcn, NB]), op=OP.mult)
                sub = rp.tile([128, TC, NB], F32, tag="sub")
                for c in (8 * math.pi, 4 * math.pi, 2 * math.pi):
                    nc.vector.tensor_scalar(out=sub[:, :tcn, :],
                                            in0=rb[:, :tcn, :],
                                            scalar1=float(c), scalar2=float(c),
                                            op0=OP.is_ge, op1=OP.mult)
                    nc.vector.tensor_tensor(out=rb[:, :tcn, :],
                                            in0=rb[:, :tcn, :],
                                            in1=sub[:, :tcn, :],
                                            op=OP.subtract)
                tau_lo = float(np.nextafter(np.float32(2 * math.pi),
                                            np.float32(0)))
                nc.vector.tensor_scalar(out=rb[:, :tcn, :], in0=rb[:, :tcn, :],
                                        scalar1=tau_lo, scalar2=None,
                                        op0=OP.min)
                nc.scalar.activation(rb[:, :tcn, :], rb[:, :tcn, :], AF.Sin,
                                     bias=-math.pi)
                rbb = rp.tile([128, TC, NB], BF16, tag="rbb")
                nc.vector.tensor_tensor(
                    out=rbb[:, :tcn, :], in0=rb[:, :tcn, :],
                    in1=co[:, :tcn, None].to_broadcast([128, tcn, NB]),
                    op=OP.mult)
                for g in range(-(-tcn // 16)):
                    lt0 = 16 * g
                    tn = min(16, tcn - lt0)
                    t0 = tc0 + lt0
                    tp = rpp.tile([128, 128], BF16, tag="tp")
                    nc.tensor.transpose(tp[:tn * NB, :],
                                        rbb[:, lt0:lt0 + tn, :], iden_s[:])
                    tsb = rw_.tile([128, 128], BF16, tag="tsb")
                    nc.vector.tensor_copy(out=tsb[:tn * NB, :],
                                          in_=tp[:tn * NB, :])
                    # dram[b, 128*(t0+t'') + e] <- tsb[NB*t'' + b, e]
                    base = rbf_dram[:]
                    dram_ap = bass.AP(base.tensor, base.offset + 128 * t0,
                                      [[128, tn], [EPC, NB], [1, 128]])
                    nc.sync.dma_start(dram_ap, tsb[:tn * NB, :])

        # ---------------- main pools ----------------
        gp = ctx.enter_context(tc.tile_pool(name="gath", bufs=3))
        mp = ctx.enter_context(tc.tile_pool(name="mlp", bufs=3))
        pp = ctx.enter_context(tc.tile_pool(name="mlpp", bufs=3, space="PSUM"))
        wpp = ctx.enter_context(tc.tile_pool(name="winp", bufs=2, space="PSUM"))
        npool = ctx.enter_context(tc.tile_pool(name="node", bufs=2))
        npp = ctx.enter_context(tc.tile_pool(name="nodep", bufs=2, space="PSUM"))
        spp = ctx.enter_context(tc.tile_pool(name="statp", bufs=1, space="PSUM"))

        def layer(l, tab_lo, tab_hi):
            def seg(t_off, wtiles, idxs, tab, first):
                cpw = wtiles // TPC
                for w in range(NW):
                    npos = wtiles * 128
                    pos0 = t_off * 128 + w * npos   # global edge position
                    spos = w * npos                 # segment-local position
                    nj = gp.tile([128, wtiles, H], BF16, tag="gat")
                    nc.gpsimd.dma_gather(
                        nj[:], tab, idxs[:, spos // 16:(spos + npos) // 16],
                        npos, npos, H, single_packet=(npos <= 1024),
                        queue_num=w % 4)
                    wps = wpp.tile([128, 128], F32, tag="wps")
                    for cc in range(cpw):
                        e0 = pos0 + cc * MLPC
                        rbfc = mp.tile([NB, MLPC], BF16, tag="rbfc")
                        nc.sync.dma_start(rbfc[:], rbf_dram[:, e0:e0 + MLPC])
                        h1p = pp.tile([128, MLPC], F32, tag="mps")
                        nc.tensor.matmul(h1p[:], lhsT=wt_s[f"w1_{l}"][:],
                                         rhs=rbfc[:], start=True, stop=True)
                        h1 = mp.tile([128, MLPC], BF16, tag="h1")
                        act_silu(mp, h1, h1p[:], wt_s[f"b1_{l}"][:, 0:1], "h1")
                        h2p = pp.tile([128, MLPC], F32, tag="mps")
                        nc.tensor.matmul(h2p[:], lhsT=wt_s[f"w2_{l}"][:],
                                         rhs=h1[:], start=True, stop=True)
                        h2 = mp.tile([128, MLPC], BF16, tag="h2")
                        act_silu(mp, h2, h2p[:], wt_s[f"b2_{l}"][:, 0:1], "h2")
                        rwp = pp.tile([128, TPC, 128], F32, tag="mps")
                        for k in range(TPC):
                            nc.tensor.matmul(rwp[:, k, :],
                                             lhsT=h2[:, 128 * k:128 * (k + 1)],
                                             rhs=wt_s[f"w3_{l}"][:],
                                             start=True, stop=True)
                        rwb = mp.tile([128, TPC, 128], BF16, tag="rwb")
                        nc.vector.tensor_tensor(
                            out=rwb[:], in0=rwp[:],
                            in1=wt_s[f"b3bc_{l}"][:].to_broadcast(
                                [128, TPC, 128]), op=OP.add)
                        j0 = cc * TPC
                        tg0 = t_off + w * wtiles + j0
                        msgs = mp.tile([128, TPC, 128], BF16, tag="msgs")
                        nc.vector.tensor_tensor(
                            out=msgs[:], in0=nj[:, j0:j0 + TPC, :],
                            in1=rwb[:], op=OP.mult)
                        sel = mp.tile([128, TPC, 128], BF16, tag="sel")
                        nc.vector.tensor_tensor(
                            out=sel[:],
                            in0=relr_s[:, tg0:tg0 + TPC, :]
                            .to_broadcast([128, TPC, 128]),
                            in1=iota_s[:]
                            .to_broadcast([128, TPC, 128]), op=OP.is_equal)
                        for k in range(TPC):
                            j = j0 + k
                            nc.tensor.matmul(wps[:], lhsT=msgs[:, k, :],
                                             rhs=sel[:, k, :],
                                             start=(j == 0),
                                             stop=(j == wtiles - 1))
                    if first:
                        nc.vector.tensor_copy(
                            out=agg[:, 128 * w:128 * (w + 1)], in_=wps[:])
                    else:
                        nc.vector.tensor_tensor(
                            out=agg[:, 128 * w:128 * (w + 1)],
                            in0=agg[:, 128 * w:128 * (w + 1)], in1=wps[:],
                            op=OP.add)

            seg(0, tl, idxl_s[:], tab_lo, True)
            seg(NSEG_L // 128, th, idxh_s[:], tab_hi, False)

            # node update + LN (feature-major); x parked in agg, stats batched
            # per group of windows (keeps the [1, *] stat tiles small)
            wg = [list(range(NW))[i:i + 17] for i in range(0, NW, 17)]
            for grp in wg:
                gn = len(grp) * 128
                mu_a = npool.tile([1, 17 * 128], F32, tag="mu_a")
                m2_a = npool.tile([1, 17 * 128], F32, tag="m2_a")
                for gi, w in enumerate(grp):
                    sl = slice(128 * w, 128 * (w + 1))
                    gsl = slice(128 * gi, 128 * (gi + 1))
                    up = npp.tile([128, 128], F32, tag="np1")
                    nc.tensor.matmul(up[:], lhsT=wt_s[f"linA_{l}"][:],
                                     rhs=feats_fm[:, sl],
                                     start=True, stop=False)
                    nc.tensor.matmul(up[:], lhsT=wt_s[f"linB_{l}"][:],
                                     rhs=agg[:, sl], start=False, stop=False)
                    nc.tensor.matmul(up[:], lhsT=wt_s[f"linb_{l}"][:],
                                     rhs=onesr_s[:], start=False, stop=True)
                    nc.vector.tensor_tensor(out=agg[:, sl], in0=up[:],
                                            in1=feats_fm[:, sl], op=OP.add)
                    x2 = npool.tile([128, 128], F32, tag="x2")
                    nc.vector.tensor_tensor(out=x2[:], in0=agg[:, sl],
                                            in1=agg[:, sl], op=OP.mult)
                    st = spp.tile([1, 256], F32, tag="st")
                    nc.tensor.matmul(st[:, 0:128], lhsT=onesm_s[:],
                                     rhs=agg[:, sl], start=True, stop=True)
                    nc.tensor.matmul(st[:, 128:256], lhsT=onesm_s[:],
                                     rhs=x2[:], start=True, stop=True)
                    nc.vector.tensor_copy(out=mu_a[:, gsl], in_=st[:, 0:128])
                    nc.vector.tensor_copy(out=m2_a[:, gsl],
                                          in_=st[:, 128:256])
                var_a = npool.tile([1, 17 * 128], F32, tag="var_a")
                nc.vector.tensor_tensor(out=var_a[:, :gn],
                                        in0=mu_a[:, :gn],
                                        in1=mu_a[:, :gn], op=OP.mult)
                nc.vector.tensor_tensor(out=var_a[:, :gn],
                                        in0=m2_a[:, :gn],
                                        in1=var_a[:, :gn], op=OP.subtract)
                nc.scalar.activation(var_a[:, :gn], var_a[:, :gn],
                                     AF.Sqrt, bias=1e-5)
                nc.vector.reciprocal(var_a[:, :gn], var_a[:, :gn])
                nc.vector.tensor_tensor(out=mu_a[:, :gn], in0=mu_a[:, :gn],
                                        in1=var_a[:, :gn], op=OP.mult)
                for gi, w in enumerate(grp):
                    sl = slice(128 * w, 128 * (w + 1))
                    gsl = slice(128 * gi, 128 * (gi + 1))
                    bc = npp.tile([128, 256], F32, tag="np1")
                    nc.tensor.matmul(bc[:, 0:128], lhsT=onesr_s[:],
                                     rhs=var_a[:, gsl], start=True, stop=True)
                    nc.tensor.matmul(bc[:, 128:256], lhsT=onesr_s[:],
                                     rhs=mu_a[:, gsl], start=True, stop=True)
                    xn = npool.tile([128, 128], F32, tag="xn")
                    nc.vector.tensor_tensor(out=xn[:], in0=agg[:, sl],
                                            in1=bc[:, 0:128], op=OP.mult)
                    nc.vector.tensor_tensor(out=xn[:], in0=xn[:],
                                            in1=bc[:, 128:256],
                                            op=OP.subtract)
                    nc.vector.tensor_tensor(
                        out=xn[:], in0=xn[:],
                        in1=wt_s[f"lng_{l}"][:].to_broadcast([128, 128]),
                        op=OP.mult)
                    nc.vector.tensor_tensor(
                        out=feats_fm[:, sl], in0=xn[:],
                        in1=wt_s[f"lnb_{l}"][:].to_broadcast([128, 128]),
                        op=OP.add)
                    if l == 0:
                        tpn = npp.tile([128, 128], F32, tag="np1")
                        nc.tensor.transpose(tpn[:], feats_fm[:, sl],
                                            idenf_s[:])
                        nm = npool.tile([128, 128], BF16, tag="nm")
                        nc.vector.tensor_copy(out=nm[:], in_=tpn[:])
                        nc.sync.dma_start(ag_in[sl, :], nm[:])

        layer(0, table0.ap()[0:SPLIT, :], table0.ap()[SPLIT:NPAD, :])
        nc.gpsimd.collective_compute(
            "AllGather", OP.bypass,
            replica_groups=[list(range(NCORES))],
            ins=[ag_in.opt()], outs=[table1.ap().opt()])
        layer(1, table1.ap()[0:SPLIT, :], table1.ap()[SPLIT:NPAD, :])

        # ---------------- readout ----------------
        er = cpool.tile([128, NW], F32, tag="er")
        for w in range(NW):
            sl = slice(128 * w, 128 * (w + 1))
            ap_ = npp.tile([128, 128], F32, tag="np1")
            nc.tensor.matmul(ap_[:], lhsT=row1_s[:], rhs=feats_fm[:, sl],
                             start=True, stop=True)
            a = npool.tile([128, 128], F32, tag="a")
            act_silu(npool, a, ap_[:], rob1_s[:, 0:1], "a")
            ep = npp.tile([128, 1], F32, tag="np1")
            nc.tensor.matmul(ep[:], lhsT=a[:], rhs=row2_s[:, 0:1],
                             start=True, stop=True)
            nc.vector.tensor_copy(out=er[:, w:w + 1], in_=ep[:])
        nc.vector.tensor_tensor(out=er[:], in0=er[:], in1=ae_s[:], op=OP.add)
        nc.vector.tensor_tensor(out=er[:], in0=er[:], in1=valid_s[:],
                                op=OP.mult)
        erd = cpool.tile([128, 1], F32, tag="erd")
        nc.vector.tensor_reduce(out=erd[:], in_=er[:],
                                axis=mybir.AxisListType.X, op=OP.add)
        tot = spp.tile([1, 1], F32, tag="st")
        nc.tensor.matmul(tot[:], lhsT=oness_s[:], rhs=erd[:],
                         start=True, stop=True)
        tsb1 = cpool.tile([1, 1], F32, tag="tsb1")
        nc.vector.tensor_copy(out=tsb1[:], in_=tot[:])
        nc.sync.dma_start(out.ap(), tsb1[:])

    nc.compile()
    return nc


# ---------------------------------------------------------------- entry
def kernel(**inputs):
    in_maps, meta, host = _prep(inputs)
    key = tuple(sorted(meta.items()))
    if key not in _CACHE:
        _CACHE[key] = _build(meta)
    nc = _CACHE[key]
    res = run_bass_kernel_spmd(nc, in_maps, core_ids=list(range(NCORES)))
    partials = [float(r["out"][0, 0]) for r in res.results]
    # device readout omits the per-node ro_b2 constant; add it for valid nodes
    total = sum(partials) + host["ro_b2"] * N
    return np.float32(total * host["scale"] + host["shift"])


# revision 41
# speedup vs baseline: 1.1505x; 1.0067x over previous
"""Trainium2 Bass kernel for nn_MACE (2-layer MACE-style GNN, scalar energy output).

Strategy (8 NeuronCores, SPMD):
  - Edges sharded by destination row range: core c owns nodes [c*6272, (c+1)*6272).
  - Full node-feature table replicated (DRAM, bf16) for the per-edge gather
    (dma_gather, int16 indices; table split at row 25088 into two halves so
    indices fit int16; edges grouped by half into two per-window segments).
  - einsum('eh,ehl->eh') trick: only sum_l rw[:, :, l] is needed, so rw_w3 is
    host-folded to [H, H] — the big radial matmul shrinks 3x.
  - Scatter (segment_sum) via one-hot matmuls: edges row-sorted into 128-node
    windows; per 128-edge tile a selection matrix sel[k, n] = (rel_row[k] == n)
    is built on DVE and PE accumulates agg windows in PSUM.
  - Node-wise linear+LN data-parallel over the core's node slice; updated
    slices are AllGathered (bf16) into the next layer's gather table.
  - Readout reduced per core to one partial scalar; host sums partials.
"""
import math
import sys
from contextlib import ExitStack

import numpy as np
import ml_dtypes

sys.path.insert(0, "/opt/trn_rl_repo")

import concourse.bacc as bacc  # noqa: E402
import concourse.bass as bass  # noqa: E402
import concourse.mybir as mybir  # noqa: E402
import concourse.tile as tile  # noqa: E402
from concourse.bass_utils import run_bass_kernel_spmd  # noqa: E402

AF = mybir.ActivationFunctionType
OP = mybir.AluOpType

N = 50000
E = 800000
H = 128
NB = 8
LMAX = 2
L = 2
CUTOFF = 5.0
NCORES = 8
NPC = 6272                # nodes per core; 8*6272 = 50176 >= N
NPAD = NCORES * NPC
NW = NPC // 128           # 49 windows per core
SPLIT = 25088             # feats table col-split (int16 index limit)
PADV = 1000.0             # rel_row value for padded edge slots (never matches iota)

F32 = mybir.dt.float32
BF16 = mybir.dt.bfloat16
I16 = mybir.dt.int16

MLPC = 384                # radial-MLP chunk: 3 tiles of 128
TPC = MLPC // 128         # tiles per chunk

bf16 = ml_dtypes.bfloat16

SIM_SILU = False   # CoreSim lacks the Silu LUT; emulate via Sigmoid + mult

_CACHE = {}


# ---------------------------------------------------------------- host prep
def _prep(inputs, force_tiles=None):
    row, col = np.asarray(inputs["edge_index"], np.int64)
    pos = np.asarray(inputs["pos"], np.float32)
    an = np.asarray(inputs["atomic_numbers"], np.int64)

    rw_w3 = np.asarray(inputs["rw_w3"], np.float32)     # [L, H, 3H]
    rw_b3 = np.asarray(inputs["rw_b3"], np.float32)     # [L, 3H]
    w3eff = rw_w3.reshape(L, H, H, LMAX + 1).sum(-1)    # [L, H, H]
    b3eff = rw_b3.reshape(L, H, LMAX + 1).sum(-1)       # [L, H]

    feats0 = np.asarray(inputs["node_emb"], np.float32)[an]          # [N, H]
    feats0 = np.concatenate([feats0, np.zeros((NPAD - N, H), np.float32)])
    ae = np.asarray(inputs["ae_emb"], np.float32)[an][:, 0]
    ae = np.concatenate([ae, np.zeros(NPAD - N, np.float32)])

    core_of = row // NPC

    percore = []
    cap_lo = cap_hi = 0
    for c in range(NCORES):
        m = core_of == c
        r = row[m] - c * NPC
        cl = col[m]
        lo = cl < SPLIT
        w = r // 128
        n_lo = np.bincount(w[lo], minlength=NW)
        n_hi = np.bincount(w[~lo], minlength=NW)
        cap_lo = max(cap_lo, int(n_lo.max()))
        cap_hi = max(cap_hi, int(n_hi.max()))
        percore.append((r, cl, lo, w))
    if force_tiles is not None:
        cap_lo = max(cap_lo, force_tiles * 128)
        cap_hi = max(cap_hi, force_tiles * 128)
    # window tile counts, multiples of TPC so MLP chunks align to windows
    tl = -(-cap_lo // 128)
    tl += (-tl) % TPC
    th = -(-cap_hi // 128)
    th += (-th) % TPC
    NSEG_L = NW * tl * 128
    NSEG_H = NW * th * 128
    EPC = NSEG_L + NSEG_H
    T = EPC // 128

    meta = dict(tl=tl, th=th, EPC=EPC, T=T, NSEG_L=NSEG_L, NSEG_H=NSEG_H)

    table0 = feats0.astype(bf16)                     # [NPAD, H]
    roots = (np.arange(1, NB + 1, dtype=np.float32) * math.pi / CUTOFF)

    in_maps = []
    for c in range(NCORES):
        r, cl, lo, w = percore[c]
        ne = len(r)
        # slot assignment: lo-windows first, then hi-windows
        base = np.where(lo, w * (tl * 128), NSEG_L + w * (th * 128))
        order = np.lexsort((np.arange(ne), base))     # stable by window-segment
        rs, cls, bases = r[order], cl[order], base[order]
        # offset within each window-segment
        off = np.arange(ne) - np.searchsorted(bases, bases, side="left")
        slot = bases + off

        filled = np.zeros(EPC, bool)
        filled[slot] = True
        src = np.zeros(EPC, np.int64)
        src[slot] = np.arange(ne)

        posr = pos[np.where(filled, (rs + c * NPC)[src], 0)]
        posc = pos[np.where(filled, np.minimum(cls[src], N - 1), 0)]
        posr[~filled] = 0.0
        posc[~filled] = 0.0
        posc[~filled, 0] = 1.0                        # pad slots get d=1

        relr = np.where(filled, (rs[src] % 128).astype(np.float32), PADV)

        gidx = np.where(filled, cls[src], 0).astype(np.int64)
        idx_lo = np.where(filled[:NSEG_L],
                          np.minimum(gidx[:NSEG_L], SPLIT - 1), 0)
        idx_hi = np.where(filled[NSEG_L:],
                          np.maximum(gidx[NSEG_L:] - SPLIT, 0), 0)

        def wrap16(ix):
            a = ix.astype(np.int16).reshape(-1, 16).T          # [16, n/16]
            return np.ascontiguousarray(np.tile(a, (8, 1)))    # [128, n/16]

        def em(x, dt=np.float32):
            # edge-major channel: slot i -> [i % 128, i // 128, ...]
            x = np.asarray(x, dt)
            tcnt = x.shape[0] // 128
            return np.ascontiguousarray(
                x.reshape(tcnt, 128, *x.shape[1:]).transpose(
                    1, 0, *range(2, x.ndim + 1)))

        nsl = c * NPC
        im = {
            "posr": em(posr), "posc": em(posc),
            "relr": em(relr, bf16).reshape(128, -1, 1).copy(),
            "idx_lo": wrap16(idx_lo), "idx_hi": wrap16(idx_hi),
            "table0": table0,
            "feats_fm0": np.ascontiguousarray(feats0[nsl:nsl + NPC].T),
            "ae_nm": np.ascontiguousarray(
                ae[nsl:nsl + NPC].reshape(NW, 128).T),
            "valid_nm": np.ascontiguousarray(
                (np.arange(nsl, nsl + NPC) < N).astype(np.float32)
                .reshape(NW, 128).T),
            "iden": np.eye(128, dtype=bf16),
            "idenf": np.eye(128, dtype=np.float32),
            "iota": np.tile(np.arange(128, dtype=bf16),
                            (128, 1)).reshape(128, 1, 128).copy(),
            "rootsc": np.tile(roots, (128, 1)).reshape(128, 1, NB).copy(),
            "cbias": np.tile(np.array([0.0, -math.pi, 1e-5], np.float32),
                             (128, 1)),
            "ones_mean": np.full((128, 1), 1.0 / H, np.float32),
            "ones_sum": np.ones((128, 1), np.float32),
            "ones_row": np.ones((1, 128), np.float32),
        }
        for l in range(L):
            im[f"w1_{l}"] = np.asarray(inputs["rw_w1"][l], bf16)
            im[f"b1_{l}"] = np.asarray(
                inputs["rw_b1"][l], np.float32).reshape(128, 1)
            im[f"w2_{l}"] = np.asarray(inputs["rw_w2"][l], bf16)
            im[f"b2_{l}"] = np.asarray(
                inputs["rw_b2"][l], np.float32).reshape(128, 1)
            im[f"w3_{l}"] = np.asarray(w3eff[l], bf16)
            im[f"b3bc_{l}"] = np.tile(
                b3eff[l], (128, 1)).reshape(128, 1, 128).astype(np.float32)
            im[f"linA_{l}"] = np.asarray(inputs["lin_w"][l][:H], np.float32)
            im[f"linB_{l}"] = np.asarray(inputs["lin_w"][l][H:], np.float32)
            im[f"linb_{l}"] = np.asarray(
                inputs["lin_b"][l], np.float32).reshape(1, 128)
            im[f"lng_{l}"] = np.asarray(
                inputs["ln_g"][l], np.float32).reshape(128, 1)
            im[f"lnb_{l}"] = np.asarray(
                inputs["ln_b"][l], np.float32).reshape(128, 1)
        im["row1"] = np.asarray(inputs["ro_w1"], np.float32)
        im["rob1"] = np.asarray(inputs["ro_b1"], np.float32).reshape(128, 1)
        im["row2"] = np.asarray(inputs["ro_w2"], np.float32)
        in_maps.append(im)

    host = dict(
        ro_b2=float(np.asarray(inputs["ro_b2"]).reshape(-1)[0]),
        scale=float(np.asarray(inputs["scale"])),
        shift=float(np.asarray(inputs["shift"])),
    )
    return in_maps, meta, host


# ---------------------------------------------------------------- program
def _build(meta):
    tl, th = meta["tl"], meta["th"]
    EPC, T = meta["EPC"], meta["T"]
    NSEG_L = meta["NSEG_L"]

    nc = bacc.Bacc("TRN2", target_bir_lowering=False, debug=False,
                   num_devices=NCORES, num_swdge_queues=4,
                   dynamic_dma_scratch_size=2 ** 15)

    def din(name, shape, dt=F32):
        return nc.dram_tensor(name, shape, dt, kind="ExternalInput")

    posr = din("posr", [128, T, 3])
    posc = din("posc", [128, T, 3])
    relr_d = din("relr", [128, T, 1], BF16)
    idx_lo = din("idx_lo", [128, NSEG_L // 16], I16)
    idx_hi = din("idx_hi", [128, (EPC - NSEG_L) // 16], I16)
    table0 = din("table0", [NPAD, H], BF16)
    feats_fm0 = din("feats_fm0", [H, NPC])
    ae_nm = din("ae_nm", [128, NW])
    valid_nm = din("valid_nm", [128, NW])
    iden = din("iden", [128, 128], BF16)
    idenf = din("idenf", [128, 128])
    iota_d = din("iota", [128, 1, 128], BF16)
    rootsc = din("rootsc", [128, 1, NB])
    cbias = din("cbias", [128, 3])
    ones_mean = din("ones_mean", [128, 1])
    ones_sum = din("ones_sum", [128, 1])
    ones_row = din("ones_row", [1, 128])

    wts = {}
    for l in range(L):
        wts[f"w1_{l}"] = din(f"w1_{l}", [NB, 128], BF16)
        wts[f"b1_{l}"] = din(f"b1_{l}", [128, 1])
        wts[f"w2_{l}"] = din(f"w2_{l}", [128, 128], BF16)
        wts[f"b2_{l}"] = din(f"b2_{l}", [128, 1])
        wts[f"w3_{l}"] = din(f"w3_{l}", [128, 128], BF16)
        wts[f"b3bc_{l}"] = din(f"b3bc_{l}", [128, 1, 128])
        wts[f"linA_{l}"] = din(f"linA_{l}", [128, 128])
        wts[f"linB_{l}"] = din(f"linB_{l}", [128, 128])
        wts[f"linb_{l}"] = din(f"linb_{l}", [1, 128])
        wts[f"lng_{l}"] = din(f"lng_{l}", [128, 1])
        wts[f"lnb_{l}"] = din(f"lnb_{l}", [128, 1])
    row1 = din("row1", [128, 128])
    rob1 = din("rob1", [128, 1])
    row2 = din("row2", [128, 1])

    out = nc.dram_tensor("out", [1, 1], F32, kind="ExternalOutput")

    with tile.TileContext(nc) as tc, ExitStack() as ctx:
        dram = ctx.enter_context(tc.tile_pool(name="dram", bufs=1, space="DRAM"))
        rbf_dram = dram.tile([NB, EPC], BF16)
        ag_in = dram.tile([NPC, H], BF16)
        table1 = nc.dram_tensor("table1", [NPAD, H], BF16, addr_space="Shared")

        cpool = ctx.enter_context(tc.tile_pool(name="consts", bufs=1))

        def cload(ap):
            t = cpool.tile(list(ap.shape), ap.dtype, tag=f"c_{ap.tensor.name}")
            nc.sync.dma_start(t[:], ap)
            return t

        relr_s = cload(relr_d.ap())
        idxl_s = cload(idx_lo.ap())
        idxh_s = cload(idx_hi.ap())
        iden_s = cload(iden.ap())
        idenf_s = cload(idenf.ap())
        iota_s = cload(iota_d.ap())
        rootsc_s = cload(rootsc.ap())
        cb_s = cload(cbias.ap())
        for ci_, cv_ in enumerate([0.0, -math.pi, 1e-5]):
            nc.const_aps.aps[(F32, cv_)] = cb_s[:, ci_:ci_ + 1]
        onesm_s = cload(ones_mean.ap())
        oness_s = cload(ones_sum.ap())
        onesr_s = cload(ones_row.ap())
        ae_s = cload(ae_nm.ap())
        valid_s = cload(valid_nm.ap())
        wt_s = {k: cload(v.ap()) for k, v in wts.items()}
        row1_s = cload(row1.ap())
        rob1_s = cload(rob1.ap())
        row2_s = cload(row2.ap())

        feats_fm = cpool.tile([H, NPC], F32, tag="feats_fm")
        nc.sync.dma_start(feats_fm[:], feats_fm0.ap())
        agg = cpool.tile([H, NPC], F32, tag="agg")

        def act_silu(pool, out_t, in_ap, bias_ap, tag):
            if not SIM_SILU:
                nc.scalar.activation(out_t[:], in_ap, AF.Silu, bias=bias_ap)
            else:
                shp = list(in_ap.shape)
                xt = pool.tile(shp, F32, tag=f"{tag}_x")
                nc.scalar.activation(xt[:], in_ap, AF.Identity, bias=bias_ap)
                sg = pool.tile(shp, F32, tag=f"{tag}_s")
                nc.scalar.activation(sg[:], xt[:], AF.Sigmoid)
                nc.vector.tensor_tensor(out=out_t[:], in0=xt[:], in1=sg[:],
                                        op=OP.mult)

        # ---------------- phase R: radial basis, once (chunked) ----------
        with tc.tile_pool(name="rbfp", bufs=1) as rp, \
             tc.tile_pool(name="rbfw", bufs=2) as rw_, \
             tc.tile_pool(name="rbfps", bufs=2, space="PSUM") as rpp:
            TC = -(-T // 2)
            for tc0 in range(0, T, TC):
                tcn = min(TC, T - tc0)
                pr = rp.tile([128, TC, 3], F32, tag="pr")
                pc = rp.tile([128, TC, 3], F32, tag="pc")
                nc.sync.dma_start(pr[:, :tcn, :], posr.ap()[:, tc0:tc0 + tcn, :])
                nc.sync.dma_start(pc[:, :tcn, :], posc.ap()[:, tc0:tc0 + tcn, :])
                dx = rp.tile([128, TC, 3], F32, tag="dx")
                nc.vector.tensor_tensor(out=dx[:, :tcn, :], in0=pc[:, :tcn, :],
                                        in1=pr[:, :tcn, :], op=OP.subtract)
                nc.vector.tensor_tensor(out=dx[:, :tcn, :], in0=dx[:, :tcn, :],
                                        in1=dx[:, :tcn, :], op=OP.mult)
                d2 = rp.tile([128, TC], F32, tag="d2")
                nc.vector.tensor_reduce(out=d2[:, :tcn], in_=dx[:, :tcn, :],
                                        axis=mybir.AxisListType.X, op=OP.add)
                dd = rp.tile([128, TC], F32, tag="dd")
                nc.scalar.activation(dd[:, :tcn], d2[:, :tcn], AF.Sqrt)
                # negated envelope: -0.5*(cos(d*pi/C)+1) = sin(d*pi/(2C))^2 - 1
                co = rp.tile([128, TC], F32, tag="co")
                nc.scalar.activation(co[:, :tcn], dd[:, :tcn], AF.Sin,
                                     scale=math.pi / (2 * CUTOFF))
                nc.scalar.activation(co[:, :tcn], co[:, :tcn], AF.Square)
                nc.vector.tensor_scalar(out=co[:, :tcn], in0=co[:, :tcn],
                                        scalar1=1.0, scalar2=None,
                                        op0=OP.subtract)
                msk = rp.tile([128, TC], F32, tag="msk")
                nc.vector.tensor_scalar(out=msk[:, :tcn], in0=dd[:, :tcn],
                                        scalar1=float(CUTOFF), scalar2=None,
                                        op0=OP.is_lt)
                nc.vector.tensor_tensor(out=co[:, :tcn], in0=co[:, :tcn],
                                        in1=msk[:, :tcn], op=OP.mult)
                dcl = rp.tile([128, TC], F32, tag="dcl")
                nc.vector.tensor_scalar(out=dcl[:, :tcn], in0=dd[:, :tcn],
                                        scalar1=1e-3, scalar2=None, op0=OP.max)
                rec = rp.tile([128, TC], F32, tag="rec")
                nc.vector.reciprocal(rec[:, :tcn], dcl[:, :tcn])
                nc.vector.tensor_tensor(out=co[:, :tcn], in0=co[:, :tcn],
                                        in1=rec[:, :tcn], op=OP.mult)
                rb = rp.tile([128, TC, NB], F32, tag="rb")
                nc.vector.tensor_tensor(
                    out=rb[:, :tcn, :],
                    in0=dd[:, :tcn, None].to_broadcast([128, tcn, NB]),
                    in1=rootsc_s[:].to_broadcast([128, tcn, NB]), op=OP.mult)
                sub = rp.tile([128, TC, NB], F32, tag="sub")
                for c in (8 * math.pi, 4 * math.pi, 2 * math.pi):
                    nc.vector.tensor_scalar(out=sub[:, :tcn, :],
                                            in0=rb[:, :tcn, :],
                                            scalar1=float(c), scalar2=float(c),
                                            op0=OP.is_ge, op1=OP.mult)
                    nc.vector.tensor_tensor(out=rb[:, :tcn, :],
                                            in0=rb[:, :tcn, :],
                                            in1=sub[:, :tcn, :],
                                            op=OP.subtract)
                tau_lo = float(np.nextafter(np.float32(2 * math.pi),
                                            np.float32(0)))
                nc.vector.tensor_scalar(out=rb[:, :tcn, :], in0=rb[:, :tcn, :],
                                        scalar1=tau_lo, scalar2=None,
                                        op0=OP.min)
                nc.scalar.activation(rb[:, :tcn, :], rb[:, :tcn, :], AF.Sin,
                                     bias=-math.pi)
                rbb = rp.tile([128, TC, NB], BF16, tag="rbb")
                nc.vector.tensor_tensor(
                    out=rbb[:, :tcn, :], in0=rb[:, :tcn, :],
                    in1=co[:, :tcn, None].to_broadcast([128, tcn, NB]),
                    op=OP.mult)
                for g in range(-(-tcn // 16)):
                    lt0 = 16 * g
                    tn = min(16, tcn - lt0)
                    t0 = tc0 + lt0
                    tp = rpp.tile([128, 128], BF16, tag="tp")
                    nc.tensor.transpose(tp[:tn * NB, :],
                                        rbb[:, lt0:lt0 + tn, :], iden_s[:])
                    tsb = rw_.tile([128, 128], BF16, tag="tsb")
                    nc.vector.tensor_copy(out=tsb[:tn * NB, :],
                                          in_=tp[:tn * NB, :])
                    # dram[b, 128*(t0+t'') + e] <- tsb[NB*t'' + b, e]
                    base = rbf_dram[:]
                    dram_ap = bass.AP(base.tensor, base.offset + 128 * t0,
                                      [[128, tn], [EPC, NB], [1, 128]])
                    nc.sync.dma_start(dram_ap, tsb[:tn * NB, :])

        # ---------------- main pools ----------------
        gp = ctx.enter_context(tc.tile_pool(name="gath", bufs=5))
        mp = ctx.enter_context(tc.tile_pool(name="mlp", bufs=3))
        pp = ctx.enter_context(tc.tile_pool(name="mlpp", bufs=3, space="PSUM"))
        wpp = ctx.enter_context(tc.tile_pool(name="winp", bufs=2, space="PSUM"))
        npool = ctx.enter_context(tc.tile_pool(name="node", bufs=2))
        npp = ctx.enter_context(tc.tile_pool(name="nodep", bufs=2, space="PSUM"))
        spp = ctx.enter_context(tc.tile_pool(name="statp", bufs=1, space="PSUM"))

        def layer(l, tab_lo, tab_hi):
            def seg(t_off, wtiles, idxs, tab, first):
                cpw = wtiles // TPC
                for w in range(NW):
                    npos = wtiles * 128
                    pos0 = t_off * 128 + w * npos   # global edge position
                    spos = w * npos                 # segment-local position
                    nj = gp.tile([128, wtiles, H], BF16, tag="gat")
                    # split across SWDGE queues: drain rings in parallel
                    nsub = wtiles // TPC
                    for sj in range(nsub):
                        sn = TPC * 128
                        s0 = spos + sj * sn
                        nc.gpsimd.dma_gather(
                            nj[:, sj * TPC:(sj + 1) * TPC, :], tab,
                            idxs[:, s0 // 16:(s0 + sn) // 16],
                            sn, sn, H, single_packet=True,
                            queue_num=(w * nsub + sj) % 4)
                    wps = wpp.tile([128, 128], F32, tag="wps")
                    for cc in range(cpw):
                        e0 = pos0 + cc * MLPC
                        rbfc = mp.tile([NB, MLPC], BF16, tag="rbfc")
                        nc.sync.dma_start(rbfc[:], rbf_dram[:, e0:e0 + MLPC])
                        h1p = pp.tile([128, MLPC], F32, tag="mps")
                        nc.tensor.matmul(h1p[:], lhsT=wt_s[f"w1_{l}"][:],
                                         rhs=rbfc[:], start=True, stop=True)
                        h1 = mp.tile([128, MLPC], BF16, tag="h1")
                        act_silu(mp, h1, h1p[:], wt_s[f"b1_{l}"][:, 0:1], "h1")
                        h2p = pp.tile([128, MLPC], F32, tag="mps")
                        nc.tensor.matmul(h2p[:], lhsT=wt_s[f"w2_{l}"][:],
                                         rhs=h1[:], start=True, stop=True)
                        h2 = mp.tile([128, MLPC], BF16, tag="h2")
                        act_silu(mp, h2, h2p[:], wt_s[f"b2_{l}"][:, 0:1], "h2")
                        rwp = pp.tile([128, TPC, 128], F32, tag="mps")
                        for k in range(TPC):
                            nc.tensor.matmul(rwp[:, k, :],
                                             lhsT=h2[:, 128 * k:128 * (k + 1)],
                                             rhs=wt_s[f"w3_{l}"][:],
                                             start=True, stop=True)
                        rwb = mp.tile([128, TPC, 128], BF16, tag="rwb")
                        nc.vector.tensor_tensor(
                            out=rwb[:], in0=rwp[:],
                            in1=wt_s[f"b3bc_{l}"][:].to_broadcast(
                                [128, TPC, 128]), op=OP.add)
                        j0 = cc * TPC
                        tg0 = t_off + w * wtiles + j0
                        msgs = mp.tile([128, TPC, 128], BF16, tag="msgs")
                        nc.vector.tensor_tensor(
                            out=msgs[:], in0=nj[:, j0:j0 + TPC, :],
                            in1=rwb[:], op=OP.mult)
                        sel = mp.tile([128, TPC, 128], BF16, tag="sel")
                        nc.vector.tensor_tensor(
                            out=sel[:],
                            in0=relr_s[:, tg0:tg0 + TPC, :]
                            .to_broadcast([128, TPC, 128]),
                            in1=iota_s[:]
                            .to_broadcast([128, TPC, 128]), op=OP.is_equal)
                        for k in range(TPC):
                            j = j0 + k
                            nc.tensor.matmul(wps[:], lhsT=msgs[:, k, :],
                                             rhs=sel[:, k, :],
                                             start=(j == 0),
                                             stop=(j == wtiles - 1))
                    if first:
                        nc.vector.tensor_copy(
                            out=agg[:, 128 * w:128 * (w + 1)], in_=wps[:])
                    else:
                        nc.vector.tensor_tensor(
                            out=agg[:, 128 * w:128 * (w + 1)],
                            in0=agg[:, 128 * w:128 * (w + 1)], in1=wps[:],
                            op=OP.add)

            seg(0, tl, idxl_s[:], tab_lo, True)
            seg(NSEG_L // 128, th, idxh_s[:], tab_hi, False)

            # node update + LN (feature-major); x parked in agg, stats batched
            # per group of windows (keeps the [1, *] stat tiles small)
            wg = [list(range(NW))[i:i + 17] for i in range(0, NW, 17)]
            for grp in wg:
                gn = len(grp) * 128
                mu_a = npool.tile([1, 17 * 128], F32, tag="mu_a")
                m2_a = npool.tile([1, 17 * 128], F32, tag="m2_a")
                for gi, w in enumerate(grp):
                    sl = slice(128 * w, 128 * (w + 1))
                    gsl = slice(128 * gi, 128 * (gi + 1))
                    up = npp.tile([128, 128], F32, tag="np1")
                    nc.tensor.matmul(up[:], lhsT=wt_s[f"linA_{l}"][:],
                                     rhs=feats_fm[:, sl],
                                     start=True, stop=False)
                    nc.tensor.matmul(up[:], lhsT=wt_s[f"linB_{l}"][:],
                                     rhs=agg[:, sl], start=False, stop=False)
                    nc.tensor.matmul(up[:], lhsT=wt_s[f"linb_{l}"][:],
                                     rhs=onesr_s[:], start=False, stop=True)
                    nc.vector.tensor_tensor(out=agg[:, sl], in0=up[:],
                                            in1=feats_fm[:, sl], op=OP.add)
                    x2 = npool.tile([128, 128], F32, tag="x2")
                    nc.vector.tensor_tensor(out=x2[:], in0=agg[:, sl],
                                            in1=agg[:, sl], op=OP.mult)
                    st = spp.tile([1, 256], F32, tag="st")
                    nc.tensor.matmul(st[:, 0:128], lhsT=onesm_s[:],
                                     rhs=agg[:, sl], start=True, stop=True)
                    nc.tensor.matmul(st[:, 128:256], lhsT=onesm_s[:],
                                     rhs=x2[:], start=True, stop=True)
                    nc.vector.tensor_copy(out=mu_a[:, gsl], in_=st[:, 0:128])
                    nc.vector.tensor_copy(out=m2_a[:, gsl],
                                          in_=st[:, 128:256])
                var_a = npool.tile([1, 17 * 128], F32, tag="var_a")
                nc.vector.tensor_tensor(out=var_a[:, :gn],
                                        in0=mu_a[:, :gn],
                                        in1=mu_a[:, :gn], op=OP.mult)
                nc.vector.tensor_tensor(out=var_a[:, :gn],
                                        in0=m2_a[:, :gn],
                                        in1=var_a[:, :gn], op=OP.subtract)
                nc.scalar.activation(var_a[:, :gn], var_a[:, :gn],
                                     AF.Sqrt, bias=1e-5)
                nc.vector.reciprocal(var_a[:, :gn], var_a[:, :gn])
                nc.vector.tensor_tensor(out=mu_a[:, :gn], in0=mu_a[:, :gn],
                                        in1=var_a[:, :gn], op=OP.mult)
                for gi, w in enumerate(grp):
                    sl = slice(128 * w, 128 * (w + 1))
                    gsl = slice(128 * gi, 128 * (gi + 1))
                    bc = npp.tile([128, 256], F32, tag="np1")
                    nc.tensor.matmul(bc[:, 0:128], lhsT=onesr_s[:],
                                     rhs=var_a[:, gsl], start=True, stop=True)
                    nc.tensor.matmul(bc[:, 128:256], lhsT=onesr_s[:],
                                     rhs=mu_a[:, gsl], start=True, stop=True)
                    xn = npool.tile([128, 128], F32, tag="xn")
                    nc.vector.tensor_tensor(out=xn[:], in0=agg[:, sl],
                                            in1=bc[:, 0:128], op=OP.mult)
                    nc.vector.tensor_tensor(out=xn[:], in0=xn[:],
                                            in1=bc[:, 128:256],
                                            op=OP.subtract)
                    nc.vector.tensor_tensor(
                        out=xn[:], in0=xn[:],
                        in1=wt_s[f"lng_{l}"][:].to_broadcast([128, 128]),
                        op=OP.mult)
                    nc.vector.tensor_tensor(
                        out=feats_fm[:, sl], in0=xn[:],
                        in1=wt_s[f"lnb_{l}"][:].to_broadcast([128, 128]),
                        op=OP.add)
                    if l == 0:
                        tpn = npp.tile([128, 128], F32, tag="np1")
                        nc.tensor.transpose(tpn[:], feats_fm[:, sl],
                                            idenf_s[:])
                        nm = npool.tile([128, 128], BF16, tag="nm")
                        nc.vector.tensor_copy(out=nm[:], in_=tpn[:])
                        nc.sync.dma_start(ag_in[sl, :], nm[:])

        layer(0, table0.ap()[0:SPLIT, :], table0.ap()[SPLIT:NPAD, :])
        nc.gpsimd.collective_compute(
            "AllGather", OP.bypass,
            replica_groups=[list(range(NCORES))],
            ins=[ag_in.opt()], outs=[table1.ap().opt()])
        layer(1, table1.ap()[0:SPLIT, :], table1.ap()[SPLIT:NPAD, :])

        # ---------------- readout ----------------
        er = cpool.tile([128, NW], F32, tag="er")
        for w in range(NW):
            sl = slice(128 * w, 128 * (w + 1))
            ap_ = npp.tile([128, 128], F32, tag="np1")
            nc.tensor.matmul(ap_[:], lhsT=row1_s[:], rhs=feats_fm[:, sl],
                             start=True, stop=True)
            a = npool.tile([128, 128], F32, tag="a")
            act_silu(npool, a, ap_[:], rob1_s[:, 0:1], "a")
            ep = npp.tile([128, 1], F32, tag="np1")
            nc.tensor.matmul(ep[:], lhsT=a[:], rhs=row2_s[:, 0:1],
                             start=True, stop=True)
            nc.vector.tensor_copy(out=er[:, w:w + 1], in_=ep[:])
        nc.vector.tensor_tensor(out=er[:], in0=er[:], in1=ae_s[:], op=OP.add)
        nc.vector.tensor_tensor(out=er[:], in0=er[:], in1=valid_s[:],
                                op=OP.mult)
        erd = cpool.tile([128, 1], F32, tag="erd")
        nc.vector.tensor_reduce(out=erd[:], in_=er[:],
                                axis=mybir.AxisListType.X, op=OP.add)
        tot = spp.tile([1, 1], F32, tag="st")
        nc.tensor.matmul(tot[:], lhsT=oness_s[:], rhs=erd[:],
                         start=True, stop=True)
        tsb1 = cpool.tile([1, 1], F32, tag="tsb1")
        nc.vector.tensor_copy(out=tsb1[:], in_=tot[:])
        nc.sync.dma_start(out.ap(), tsb1[:])

    nc.compile()
    return nc


# ---------------------------------------------------------------- entry
def kernel(**inputs):
    in_maps, meta, host = _prep(inputs)
    key = tuple(sorted(meta.items()))
    if key not in _CACHE:
        _CACHE[key] = _build(meta)
    nc = _CACHE[key]
    res = run_bass_kernel_spmd(nc, in_maps, core_ids=list(range(NCORES)))
    partials = [float(r["out"][0, 0]) for r in res.results]
    # device readout omits the per-node ro_b2 constant; add it for valid nodes
    total = sum(partials) + host["ro_b2"] * N
    return np.float32(total * host["scale"] + host["shift"])


# revision 42
# speedup vs baseline: 1.4592x; 1.2683x over previous
"""Trainium2 Bass kernel for nn_MACE (2-layer MACE-style GNN, scalar energy output).

Strategy (8 NeuronCores, SPMD):
  - Edges sharded by destination row range: core c owns nodes [c*6272, (c+1)*6272).
  - Full node-feature table replicated (DRAM, bf16) for the per-edge gather
    (dma_gather, int16 indices; table split at row 25088 into two halves so
    indices fit int16; edges grouped by half into two per-window segments).
  - einsum('eh,ehl->eh') trick: only sum_l rw[:, :, l] is needed, so rw_w3 is
    host-folded to [H, H] — the big radial matmul shrinks 3x.
  - Scatter (segment_sum) via one-hot matmuls: edges row-sorted into 128-node
    windows; per 128-edge tile a selection matrix sel[k, n] = (rel_row[k] == n)
    is built on DVE and PE accumulates agg windows in PSUM.
  - Node-wise linear+LN data-parallel over the core's node slice; updated
    slices are AllGathered (bf16) into the next layer's gather table.
  - Readout reduced per core to one partial scalar; host sums partials.
"""
import math
import sys
from contextlib import ExitStack

import numpy as np
import ml_dtypes

sys.path.insert(0, "/opt/trn_rl_repo")

import concourse.bacc as bacc  # noqa: E402
import concourse.bass as bass  # noqa: E402
import concourse.mybir as mybir  # noqa: E402
import concourse.tile as tile  # noqa: E402
from concourse.bass_utils import run_bass_kernel_spmd  # noqa: E402

AF = mybir.ActivationFunctionType
OP = mybir.AluOpType

N = 50000
E = 800000
H = 128
NB = 8
LMAX = 2
L = 2
CUTOFF = 5.0
NCORES = 8
NPC = 6272                # nodes per core; 8*6272 = 50176 >= N
NPAD = NCORES * NPC
NW = NPC // 128           # 49 windows per core
SPLIT = 25088             # feats table col-split (int16 index limit)
PADV = 1000.0             # rel_row value for padded edge slots (never matches iota)

F32 = mybir.dt.float32
BF16 = mybir.dt.bfloat16
I16 = mybir.dt.int16

MLPC = 384                # radial-MLP chunk: 3 tiles of 128
TPC = MLPC // 128         # tiles per chunk

bf16 = ml_dtypes.bfloat16

SIM_SILU = False   # CoreSim lacks the Silu LUT; emulate via Sigmoid + mult

_CACHE = {}


# ---------------------------------------------------------------- host prep
def _prep(inputs, force_tiles=None):
    row, col = np.asarray(inputs["edge_index"], np.int64)
    pos = np.asarray(inputs["pos"], np.float32)
    an = np.asarray(inputs["atomic_numbers"], np.int64)

    rw_w3 = np.asarray(inputs["rw_w3"], np.float32)     # [L, H, 3H]
    rw_b3 = np.asarray(inputs["rw_b3"], np.float32)     # [L, 3H]
    w3eff = rw_w3.reshape(L, H, H, LMAX + 1).sum(-1)    # [L, H, H]
    b3eff = rw_b3.reshape(L, H, LMAX + 1).sum(-1)       # [L, H]

    feats0 = np.asarray(inputs["node_emb"], np.float32)[an]          # [N, H]
    feats0 = np.concatenate([feats0, np.zeros((NPAD - N, H), np.float32)])
    ae = np.asarray(inputs["ae_emb"], np.float32)[an][:, 0]
    ae = np.concatenate([ae, np.zeros(NPAD - N, np.float32)])

    core_of = row // NPC

    percore = []
    cap_lo = cap_hi = 0
    for c in range(NCORES):
        m = core_of == c
        r = row[m] - c * NPC
        cl = col[m]
        lo = cl < SPLIT
        w = r // 128
        n_lo = np.bincount(w[lo], minlength=NW)
        n_hi = np.bincount(w[~lo], minlength=NW)
        cap_lo = max(cap_lo, int(n_lo.max()))
        cap_hi = max(cap_hi, int(n_hi.max()))
        percore.append((r, cl, lo, w))
    if force_tiles is not None:
        cap_lo = max(cap_lo, force_tiles * 128)
        cap_hi = max(cap_hi, force_tiles * 128)
    # window tile counts, multiples of TPC so MLP chunks align to windows
    tl = -(-cap_lo // 128)
    tl += (-tl) % TPC
    th = -(-cap_hi // 128)
    th += (-th) % TPC
    NSEG_L = NW * tl * 128
    NSEG_H = NW * th * 128
    EPC = NSEG_L + NSEG_H
    T = EPC // 128

    meta = dict(tl=tl, th=th, EPC=EPC, T=T, NSEG_L=NSEG_L, NSEG_H=NSEG_H)

    table0 = feats0.astype(bf16)                     # [NPAD, H]
    roots = (np.arange(1, NB + 1, dtype=np.float32) * math.pi / CUTOFF)

    in_maps = []
    for c in range(NCORES):
        r, cl, lo, w = percore[c]
        ne = len(r)
        # slot assignment: lo-windows first, then hi-windows
        base = np.where(lo, w * (tl * 128), NSEG_L + w * (th * 128))
        order = np.lexsort((np.arange(ne), base))     # stable by window-segment
        rs, cls, bases = r[order], cl[order], base[order]
        # offset within each window-segment
        off = np.arange(ne) - np.searchsorted(bases, bases, side="left")
        slot = bases + off

        filled = np.zeros(EPC, bool)
        filled[slot] = True
        src = np.zeros(EPC, np.int64)
        src[slot] = np.arange(ne)

        posr = pos[np.where(filled, (rs + c * NPC)[src], 0)]
        posc = pos[np.where(filled, np.minimum(cls[src], N - 1), 0)]
        posr[~filled] = 0.0
        posc[~filled] = 0.0
        posc[~filled, 0] = 1.0                        # pad slots get d=1

        relr = np.where(filled, (rs[src] % 128).astype(np.float32), PADV)

        gidx = np.where(filled, cls[src], 0).astype(np.int64)
        idx_lo = np.where(filled[:NSEG_L],
                          np.minimum(gidx[:NSEG_L], SPLIT - 1), 0)
        idx_hi = np.where(filled[NSEG_L:],
                          np.maximum(gidx[NSEG_L:] - SPLIT, 0), 0)

        def wrap16(ix):
            a = ix.astype(np.int16).reshape(-1, 16).T          # [16, n/16]
            return np.ascontiguousarray(np.tile(a, (8, 1)))    # [128, n/16]

        def em(x, dt=np.float32):
            # edge-major channel: slot i -> [i % 128, i // 128, ...]
            x = np.asarray(x, dt)
            tcnt = x.shape[0] // 128
            return np.ascontiguousarray(
                x.reshape(tcnt, 128, *x.shape[1:]).transpose(
                    1, 0, *range(2, x.ndim + 1)))

        nsl = c * NPC
        im = {
            "posr": em(posr), "posc": em(posc),
            "relr": em(relr, bf16).reshape(128, -1, 1).copy(),
            "idx_lo": wrap16(idx_lo), "idx_hi": wrap16(idx_hi),
            "table0": table0,
            "feats_fm0": np.ascontiguousarray(feats0[nsl:nsl + NPC].T),
            "ae_nm": np.ascontiguousarray(
                ae[nsl:nsl + NPC].reshape(NW, 128).T),
            "valid_nm": np.ascontiguousarray(
                (np.arange(nsl, nsl + NPC) < N).astype(np.float32)
                .reshape(NW, 128).T),
            "iden": np.eye(128, dtype=bf16),
            "idenf": np.eye(128, dtype=np.float32),
            "iota": np.tile(np.arange(128, dtype=bf16),
                            (128, 1)).reshape(128, 1, 128).copy(),
            "rootsc": np.tile(roots, (128, 1)).reshape(128, 1, NB).copy(),
            "cbias": np.tile(np.array([0.0, -math.pi, 1e-5], np.float32),
                             (128, 1)),
            "ones_mean": np.full((128, 1), 1.0 / H, np.float32),
            "ones_sum": np.ones((128, 1), np.float32),
            "ones_row": np.ones((1, 128), np.float32),
        }
        for l in range(L):
            im[f"w1_{l}"] = np.asarray(inputs["rw_w1"][l], bf16)
            im[f"b1_{l}"] = np.asarray(
                inputs["rw_b1"][l], np.float32).reshape(128, 1)
            im[f"w2_{l}"] = np.asarray(inputs["rw_w2"][l], bf16)
            im[f"b2_{l}"] = np.asarray(
                inputs["rw_b2"][l], np.float32).reshape(128, 1)
            im[f"w3_{l}"] = np.asarray(w3eff[l], bf16)
            im[f"b3bc_{l}"] = np.tile(
                b3eff[l], (128, 1)).reshape(128, 1, 128).astype(np.float32)
            im[f"linA_{l}"] = np.asarray(inputs["lin_w"][l][:H], np.float32)
            im[f"linB_{l}"] = np.asarray(inputs["lin_w"][l][H:], np.float32)
            im[f"linb_{l}"] = np.asarray(
                inputs["lin_b"][l], np.float32).reshape(1, 128)
            im[f"lng_{l}"] = np.asarray(
                inputs["ln_g"][l], np.float32).reshape(128, 1)
            im[f"lnb_{l}"] = np.asarray(
                inputs["ln_b"][l], np.float32).reshape(128, 1)
        im["row1"] = np.asarray(inputs["ro_w1"], np.float32)
        im["rob1"] = np.asarray(inputs["ro_b1"], np.float32).reshape(128, 1)
        im["row2"] = np.asarray(inputs["ro_w2"], np.float32)
        in_maps.append(im)

    host = dict(
        ro_b2=float(np.asarray(inputs["ro_b2"]).reshape(-1)[0]),
        scale=float(np.asarray(inputs["scale"])),
        shift=float(np.asarray(inputs["shift"])),
    )
    return in_maps, meta, host


# ---------------------------------------------------------------- program
def _build(meta):
    tl, th = meta["tl"], meta["th"]
    EPC, T = meta["EPC"], meta["T"]
    NSEG_L = meta["NSEG_L"]

    nc = bacc.Bacc("TRN2", target_bir_lowering=False, debug=False,
                   num_devices=NCORES, num_swdge_queues=4,
                   dynamic_dma_scratch_size=2 ** 15)

    def din(name, shape, dt=F32):
        return nc.dram_tensor(name, shape, dt, kind="ExternalInput")

    posr = din("posr", [128, T, 3])
    posc = din("posc", [128, T, 3])
    relr_d = din("relr", [128, T, 1], BF16)
    idx_lo = din("idx_lo", [128, NSEG_L // 16], I16)
    idx_hi = din("idx_hi", [128, (EPC - NSEG_L) // 16], I16)
    table0 = din("table0", [NPAD, H], BF16)
    feats_fm0 = din("feats_fm0", [H, NPC])
    ae_nm = din("ae_nm", [128, NW])
    valid_nm = din("valid_nm", [128, NW])
    iden = din("iden", [128, 128], BF16)
    idenf = din("idenf", [128, 128])
    iota_d = din("iota", [128, 1, 128], BF16)
    rootsc = din("rootsc", [128, 1, NB])
    cbias = din("cbias", [128, 3])
    ones_mean = din("ones_mean", [128, 1])
    ones_sum = din("ones_sum", [128, 1])
    ones_row = din("ones_row", [1, 128])

    wts = {}
    for l in range(L):
        wts[f"w1_{l}"] = din(f"w1_{l}", [NB, 128], BF16)
        wts[f"b1_{l}"] = din(f"b1_{l}", [128, 1])
        wts[f"w2_{l}"] = din(f"w2_{l}", [128, 128], BF16)
        wts[f"b2_{l}"] = din(f"b2_{l}", [128, 1])
        wts[f"w3_{l}"] = din(f"w3_{l}", [128, 128], BF16)
        wts[f"b3bc_{l}"] = din(f"b3bc_{l}", [128, 1, 128])
        wts[f"linA_{l}"] = din(f"linA_{l}", [128, 128])
        wts[f"linB_{l}"] = din(f"linB_{l}", [128, 128])
        wts[f"linb_{l}"] = din(f"linb_{l}", [1, 128])
        wts[f"lng_{l}"] = din(f"lng_{l}", [128, 1])
        wts[f"lnb_{l}"] = din(f"lnb_{l}", [128, 1])
    row1 = din("row1", [128, 128])
    rob1 = din("rob1", [128, 1])
    row2 = din("row2", [128, 1])

    out = nc.dram_tensor("out", [1, 1], F32, kind="ExternalOutput")

    with tile.TileContext(nc) as tc, ExitStack() as ctx:
        dram = ctx.enter_context(tc.tile_pool(name="dram", bufs=1, space="DRAM"))
        rbf_dram = dram.tile([NB, EPC], BF16)
        ag_in = dram.tile([NPC, H], BF16)
        table1 = nc.dram_tensor("table1", [NPAD, H], BF16, addr_space="Shared")

        cpool = ctx.enter_context(tc.tile_pool(name="consts", bufs=1))

        def cload(ap):
            t = cpool.tile(list(ap.shape), ap.dtype, tag=f"c_{ap.tensor.name}")
            nc.sync.dma_start(t[:], ap)
            return t

        relr_s = cload(relr_d.ap())
        idxl_s = cload(idx_lo.ap())
        idxh_s = cload(idx_hi.ap())
        iden_s = cload(iden.ap())
        idenf_s = cload(idenf.ap())
        iota_s = cload(iota_d.ap())
        rootsc_s = cload(rootsc.ap())
        cb_s = cload(cbias.ap())
        for ci_, cv_ in enumerate([0.0, -math.pi, 1e-5]):
            nc.const_aps.aps[(F32, cv_)] = cb_s[:, ci_:ci_ + 1]
        onesm_s = cload(ones_mean.ap())
        oness_s = cload(ones_sum.ap())
        onesr_s = cload(ones_row.ap())
        ae_s = cload(ae_nm.ap())
        valid_s = cload(valid_nm.ap())
        wt_s = {k: cload(v.ap()) for k, v in wts.items()}
        row1_s = cload(row1.ap())
        rob1_s = cload(rob1.ap())
        row2_s = cload(row2.ap())

        feats_fm = cpool.tile([H, NPC], F32, tag="feats_fm")
        nc.sync.dma_start(feats_fm[:], feats_fm0.ap())
        agg = cpool.tile([H, NPC], F32, tag="agg")

        def act_silu(pool, out_t, in_ap, bias_ap, tag):
            if not SIM_SILU:
                nc.scalar.activation(out_t[:], in_ap, AF.Silu, bias=bias_ap)
            else:
                shp = list(in_ap.shape)
                xt = pool.tile(shp, F32, tag=f"{tag}_x")
                nc.scalar.activation(xt[:], in_ap, AF.Identity, bias=bias_ap)
                sg = pool.tile(shp, F32, tag=f"{tag}_s")
                nc.scalar.activation(sg[:], xt[:], AF.Sigmoid)
                nc.vector.tensor_tensor(out=out_t[:], in0=xt[:], in1=sg[:],
                                        op=OP.mult)

        # ---------------- phase R: radial basis, once (chunked) ----------
        with tc.tile_pool(name="rbfp", bufs=1) as rp, \
             tc.tile_pool(name="rbfw", bufs=2) as rw_, \
             tc.tile_pool(name="rbfps", bufs=2, space="PSUM") as rpp:
            TC = -(-T // 2)
            for tc0 in range(0, T, TC):
                tcn = min(TC, T - tc0)
                pr = rp.tile([128, TC, 3], F32, tag="pr")
                pc = rp.tile([128, TC, 3], F32, tag="pc")
                nc.sync.dma_start(pr[:, :tcn, :], posr.ap()[:, tc0:tc0 + tcn, :])
                nc.sync.dma_start(pc[:, :tcn, :], posc.ap()[:, tc0:tc0 + tcn, :])
                dx = rp.tile([128, TC, 3], F32, tag="dx")
                nc.vector.tensor_tensor(out=dx[:, :tcn, :], in0=pc[:, :tcn, :],
                                        in1=pr[:, :tcn, :], op=OP.subtract)
                nc.vector.tensor_tensor(out=dx[:, :tcn, :], in0=dx[:, :tcn, :],
                                        in1=dx[:, :tcn, :], op=OP.mult)
                d2 = rp.tile([128, TC], F32, tag="d2")
                nc.vector.tensor_reduce(out=d2[:, :tcn], in_=dx[:, :tcn, :],
                                        axis=mybir.AxisListType.X, op=OP.add)
                dd = rp.tile([128, TC], F32, tag="dd")
                nc.scalar.activation(dd[:, :tcn], d2[:, :tcn], AF.Sqrt)
                # negated envelope: -0.5*(cos(d*pi/C)+1) = sin(d*pi/(2C))^2 - 1
                co = rp.tile([128, TC], F32, tag="co")
                nc.scalar.activation(co[:, :tcn], dd[:, :tcn], AF.Sin,
                                     scale=math.pi / (2 * CUTOFF))
                nc.scalar.activation(co[:, :tcn], co[:, :tcn], AF.Square)
                nc.vector.tensor_scalar(out=co[:, :tcn], in0=co[:, :tcn],
                                        scalar1=1.0, scalar2=None,
                                        op0=OP.subtract)
                msk = rp.tile([128, TC], F32, tag="msk")
                nc.vector.tensor_scalar(out=msk[:, :tcn], in0=dd[:, :tcn],
                                        scalar1=float(CUTOFF), scalar2=None,
                                        op0=OP.is_lt)
                nc.vector.tensor_tensor(out=co[:, :tcn], in0=co[:, :tcn],
                                        in1=msk[:, :tcn], op=OP.mult)
                dcl = rp.tile([128, TC], F32, tag="dcl")
                nc.vector.tensor_scalar(out=dcl[:, :tcn], in0=dd[:, :tcn],
                                        scalar1=1e-3, scalar2=None, op0=OP.max)
                rec = rp.tile([128, TC], F32, tag="rec")
                nc.vector.reciprocal(rec[:, :tcn], dcl[:, :tcn])
                nc.vector.tensor_tensor(out=co[:, :tcn], in0=co[:, :tcn],
                                        in1=rec[:, :tcn], op=OP.mult)
                rb = rp.tile([128, TC, NB], F32, tag="rb")
                nc.vector.tensor_tensor(
                    out=rb[:, :tcn, :],
                    in0=dd[:, :tcn, None].to_broadcast([128, tcn, NB]),
                    in1=rootsc_s[:].to_broadcast([128, tcn, NB]), op=OP.mult)
                sub = rp.tile([128, TC, NB], F32, tag="sub")
                for c in (8 * math.pi, 4 * math.pi, 2 * math.pi):
                    nc.vector.tensor_scalar(out=sub[:, :tcn, :],
                                            in0=rb[:, :tcn, :],
                                            scalar1=float(c), scalar2=float(c),
                                            op0=OP.is_ge, op1=OP.mult)
                    nc.vector.tensor_tensor(out=rb[:, :tcn, :],
                                            in0=rb[:, :tcn, :],
                                            in1=sub[:, :tcn, :],
                                            op=OP.subtract)
                tau_lo = float(np.nextafter(np.float32(2 * math.pi),
                                            np.float32(0)))
                nc.vector.tensor_scalar(out=rb[:, :tcn, :], in0=rb[:, :tcn, :],
                                        scalar1=tau_lo, scalar2=None,
                                        op0=OP.min)
                nc.scalar.activation(rb[:, :tcn, :], rb[:, :tcn, :], AF.Sin,
                                     bias=-math.pi)
                rbb = rp.tile([128, TC, NB], BF16, tag="rbb")
                nc.vector.tensor_tensor(
                    out=rbb[:, :tcn, :], in0=rb[:, :tcn, :],
                    in1=co[:, :tcn, None].to_broadcast([128, tcn, NB]),
                    op=OP.mult)
                for g in range(-(-tcn // 16)):
                    lt0 = 16 * g
                    tn = min(16, tcn - lt0)
                    t0 = tc0 + lt0
                    tp = rpp.tile([128, 128], BF16, tag="tp")
                    nc.tensor.transpose(tp[:tn * NB, :],
                                        rbb[:, lt0:lt0 + tn, :], iden_s[:])
                    tsb = rw_.tile([128, 128], BF16, tag="tsb")
                    nc.vector.tensor_copy(out=tsb[:tn * NB, :],
                                          in_=tp[:tn * NB, :])
                    # dram[b, 128*(t0+t'') + e] <- tsb[NB*t'' + b, e]
                    base = rbf_dram[:]
                    dram_ap = bass.AP(base.tensor, base.offset + 128 * t0,
                                      [[128, tn], [EPC, NB], [1, 128]])
                    nc.sync.dma_start(dram_ap, tsb[:tn * NB, :])

        # ---------------- main pools ----------------
        gp = ctx.enter_context(tc.tile_pool(name="gath", bufs=5))
        mp = ctx.enter_context(tc.tile_pool(name="mlp", bufs=4))
        pp = ctx.enter_context(tc.tile_pool(name="mlpp", bufs=2, space="PSUM"))
        wpp = ctx.enter_context(tc.tile_pool(name="winp", bufs=2, space="PSUM"))
        npool = ctx.enter_context(tc.tile_pool(name="node", bufs=2))
        npp = pp
        spp = pp

        def layer(l, tab_lo, tab_hi):
            def seg(t_off, wtiles, idxs, tab, first):
                cpw = wtiles // TPC
                for w in range(NW):
                    npos = wtiles * 128
                    pos0 = t_off * 128 + w * npos   # global edge position
                    spos = w * npos                 # segment-local position
                    nj = gp.tile([128, wtiles, H], BF16, tag="gat")
                    # split across SWDGE queues: drain rings in parallel
                    nsub = wtiles // TPC
                    for sj in range(nsub):
                        sn = TPC * 128
                        s0 = spos + sj * sn
                        nc.gpsimd.dma_gather(
                            nj[:, sj * TPC:(sj + 1) * TPC, :], tab,
                            idxs[:, s0 // 16:(s0 + sn) // 16],
                            sn, sn, H, single_packet=True,
                            queue_num=(w * nsub + sj) % 4)
                    wps = wpp.tile([128, 128], F32, tag="wps")
                    for cc in range(cpw):
                        e0 = pos0 + cc * MLPC
                        rbfc = mp.tile([NB, MLPC], BF16, tag="rbfc")
                        nc.sync.dma_start(rbfc[:], rbf_dram[:, e0:e0 + MLPC])
                        h1p = pp.tile([128, MLPC], F32, tag="h1p")
                        nc.tensor.matmul(h1p[:], lhsT=wt_s[f"w1_{l}"][:],
                                         rhs=rbfc[:], start=True, stop=True)
                        h1 = mp.tile([128, MLPC], BF16, tag="h1")
                        act_silu(mp, h1, h1p[:], wt_s[f"b1_{l}"][:, 0:1], "h1")
                        h2p = pp.tile([128, MLPC], F32, tag="h2p")
                        nc.tensor.matmul(h2p[:], lhsT=wt_s[f"w2_{l}"][:],
                                         rhs=h1[:], start=True, stop=True)
                        h2 = mp.tile([128, MLPC], BF16, tag="h2")
                        act_silu(mp, h2, h2p[:], wt_s[f"b2_{l}"][:, 0:1], "h2")
                        rwp = pp.tile([128, TPC, 128], F32, tag="rwp")
                        for k in range(TPC):
                            nc.tensor.matmul(rwp[:, k, :],
                                             lhsT=h2[:, 128 * k:128 * (k + 1)],
                                             rhs=wt_s[f"w3_{l}"][:],
                                             start=True, stop=True)
                        rwb = mp.tile([128, TPC, 128], BF16, tag="rwb")
                        nc.vector.tensor_tensor(
                            out=rwb[:], in0=rwp[:],
                            in1=wt_s[f"b3bc_{l}"][:].to_broadcast(
                                [128, TPC, 128]), op=OP.add)
                        j0 = cc * TPC
                        tg0 = t_off + w * wtiles + j0
                        msgs = mp.tile([128, TPC, 128], BF16, tag="msgs")
                        nc.vector.tensor_tensor(
                            out=msgs[:], in0=nj[:, j0:j0 + TPC, :],
                            in1=rwb[:], op=OP.mult)
                        sel = mp.tile([128, TPC, 128], BF16, tag="sel")
                        nc.vector.tensor_tensor(
                            out=sel[:],
                            in0=relr_s[:, tg0:tg0 + TPC, :]
                            .to_broadcast([128, TPC, 128]),
                            in1=iota_s[:]
                            .to_broadcast([128, TPC, 128]), op=OP.is_equal)
                        for k in range(TPC):
                            j = j0 + k
                            nc.tensor.matmul(wps[:], lhsT=msgs[:, k, :],
                                             rhs=sel[:, k, :],
                                             start=(j == 0),
                                             stop=(j == wtiles - 1))
                    if first:
                        nc.vector.tensor_copy(
                            out=agg[:, 128 * w:128 * (w + 1)], in_=wps[:])
                    else:
                        nc.vector.tensor_tensor(
                            out=agg[:, 128 * w:128 * (w + 1)],
                            in0=agg[:, 128 * w:128 * (w + 1)], in1=wps[:],
                            op=OP.add)

            seg(0, tl, idxl_s[:], tab_lo, True)
            seg(NSEG_L // 128, th, idxh_s[:], tab_hi, False)

            # node update + LN (feature-major); x parked in agg, stats batched
            # per group of windows (keeps the [1, *] stat tiles small)
            wg = [list(range(NW))[i:i + 17] for i in range(0, NW, 17)]
            for grp in wg:
                gn = len(grp) * 128
                mu_a = npool.tile([1, 17 * 128], F32, tag="mu_a")
                m2_a = npool.tile([1, 17 * 128], F32, tag="m2_a")
                for gi, w in enumerate(grp):
                    sl = slice(128 * w, 128 * (w + 1))
                    gsl = slice(128 * gi, 128 * (gi + 1))
                    up = npp.tile([128, 128], F32, tag="h1p")
                    nc.tensor.matmul(up[:], lhsT=wt_s[f"linA_{l}"][:],
                                     rhs=feats_fm[:, sl],
                                     start=True, stop=False)
                    nc.tensor.matmul(up[:], lhsT=wt_s[f"linB_{l}"][:],
                                     rhs=agg[:, sl], start=False, stop=False)
                    nc.tensor.matmul(up[:], lhsT=wt_s[f"linb_{l}"][:],
                                     rhs=onesr_s[:], start=False, stop=True)
                    nc.vector.tensor_tensor(out=agg[:, sl], in0=up[:],
                                            in1=feats_fm[:, sl], op=OP.add)
                    x2 = npool.tile([128, 128], F32, tag="x2")
                    nc.vector.tensor_tensor(out=x2[:], in0=agg[:, sl],
                                            in1=agg[:, sl], op=OP.mult)
                    st = spp.tile([1, 256], F32, tag="h2p")
                    nc.tensor.matmul(st[:, 0:128], lhsT=onesm_s[:],
                                     rhs=agg[:, sl], start=True, stop=True)
                    nc.tensor.matmul(st[:, 128:256], lhsT=onesm_s[:],
                                     rhs=x2[:], start=True, stop=True)
                    nc.vector.tensor_copy(out=mu_a[:, gsl], in_=st[:, 0:128])
                    nc.vector.tensor_copy(out=m2_a[:, gsl],
                                          in_=st[:, 128:256])
                var_a = npool.tile([1, 17 * 128], F32, tag="var_a")
                nc.vector.tensor_tensor(out=var_a[:, :gn],
                                        in0=mu_a[:, :gn],
                                        in1=mu_a[:, :gn], op=OP.mult)
                nc.vector.tensor_tensor(out=var_a[:, :gn],
                                        in0=m2_a[:, :gn],
                                        in1=var_a[:, :gn], op=OP.subtract)
                nc.scalar.activation(var_a[:, :gn], var_a[:, :gn],
                                     AF.Sqrt, bias=1e-5)
                nc.vector.reciprocal(var_a[:, :gn], var_a[:, :gn])
                nc.vector.tensor_tensor(out=mu_a[:, :gn], in0=mu_a[:, :gn],
                                        in1=var_a[:, :gn], op=OP.mult)
                for gi, w in enumerate(grp):
                    sl = slice(128 * w, 128 * (w + 1))
                    gsl = slice(128 * gi, 128 * (gi + 1))
                    bc = npp.tile([128, 256], F32, tag="rwp")
                    nc.tensor.matmul(bc[:, 0:128], lhsT=onesr_s[:],
                                     rhs=var_a[:, gsl], start=True, stop=True)
                    nc.tensor.matmul(bc[:, 128:256], lhsT=onesr_s[:],
                                     rhs=mu_a[:, gsl], start=True, stop=True)
                    xn = npool.tile([128, 128], F32, tag="xn")
                    nc.vector.tensor_tensor(out=xn[:], in0=agg[:, sl],
                                            in1=bc[:, 0:128], op=OP.mult)
                    nc.vector.tensor_tensor(out=xn[:], in0=xn[:],
                                            in1=bc[:, 128:256],
                                            op=OP.subtract)
                    nc.vector.tensor_tensor(
                        out=xn[:], in0=xn[:],
                        in1=wt_s[f"lng_{l}"][:].to_broadcast([128, 128]),
                        op=OP.mult)
                    nc.vector.tensor_tensor(
                        out=feats_fm[:, sl], in0=xn[:],
                        in1=wt_s[f"lnb_{l}"][:].to_broadcast([128, 128]),
                        op=OP.add)
                    if l == 0:
                        tpn = npp.tile([128, 128], F32, tag="h1p")
                        nc.tensor.transpose(tpn[:], feats_fm[:, sl],
                                            idenf_s[:])
                        nm = npool.tile([128, 128], BF16, tag="nm")
                        nc.vector.tensor_copy(out=nm[:], in_=tpn[:])
                        nc.sync.dma_start(ag_in[sl, :], nm[:])

        layer(0, table0.ap()[0:SPLIT, :], table0.ap()[SPLIT:NPAD, :])
        nc.gpsimd.collective_compute(
            "AllGather", OP.bypass,
            replica_groups=[list(range(NCORES))],
            ins=[ag_in.opt()], outs=[table1.ap().opt()])
        layer(1, table1.ap()[0:SPLIT, :], table1.ap()[SPLIT:NPAD, :])

        # ---------------- readout ----------------
        er = cpool.tile([128, NW], F32, tag="er")
        for w in range(NW):
            sl = slice(128 * w, 128 * (w + 1))
            ap_ = npp.tile([128, 128], F32, tag="h1p")
            nc.tensor.matmul(ap_[:], lhsT=row1_s[:], rhs=feats_fm[:, sl],
                             start=True, stop=True)
            a = npool.tile([128, 128], F32, tag="a")
            act_silu(npool, a, ap_[:], rob1_s[:, 0:1], "a")
            ep = npp.tile([128, 1], F32, tag="rwp")
            nc.tensor.matmul(ep[:], lhsT=a[:], rhs=row2_s[:, 0:1],
                             start=True, stop=True)
            nc.vector.tensor_copy(out=er[:, w:w + 1], in_=ep[:])
        nc.vector.tensor_tensor(out=er[:], in0=er[:], in1=ae_s[:], op=OP.add)
        nc.vector.tensor_tensor(out=er[:], in0=er[:], in1=valid_s[:],
                                op=OP.mult)
        erd = cpool.tile([128, 1], F32, tag="erd")
        nc.vector.tensor_reduce(out=erd[:], in_=er[:],
                                axis=mybir.AxisListType.X, op=OP.add)
        tot = spp.tile([1, 1], F32, tag="h2p")
        nc.tensor.matmul(tot[:], lhsT=oness_s[:], rhs=erd[:],
                         start=True, stop=True)
        tsb1 = cpool.tile([1, 1], F32, tag="tsb1")
        nc.vector.tensor_copy(out=tsb1[:], in_=tot[:])
        nc.sync.dma_start(out.ap(), tsb1[:])

    nc.compile()
    return nc


# ---------------------------------------------------------------- entry
def kernel(**inputs):
    in_maps, meta, host = _prep(inputs)
    key = tuple(sorted(meta.items()))
    if key not in _CACHE:
        _CACHE[key] = _build(meta)
    nc = _CACHE[key]
    res = run_bass_kernel_spmd(nc, in_maps, core_ids=list(range(NCORES)))
    partials = [float(r["out"][0, 0]) for r in res.results]
    # device readout omits the per-node ro_b2 constant; add it for valid nodes
    total = sum(partials) + host["ro_b2"] * N
    return np.float32(total * host["scale"] + host["shift"])
